# revision 16
# baseline (speedup 1.0000x reference)
"""MoE language model (2-layer transformer, top-2-of-8 MoE, 32k vocab projection)
distributed over 8 TRN2 NeuronCores.

Sharding:
  - attention: head-parallel (2 of 16 heads per core) + token-parallel epilogues
  - MoE: expert-parallel (1 expert per core), dense over tokens, combine via
    ReduceScatter of comb-weighted per-expert outputs
  - vocab projection: column-parallel (4000 of 32000 cols per core)
  - residual stream: token-sharded (256 tokens per core); AllGathers produce the
    replicated feature-major activations each matmul needs.

All matmuls run in bf16 (fp32 PSUM accumulation); everything else fp32.
"""

import math
from contextlib import ExitStack

import numpy as np
import ml_dtypes

import concourse.bass as bass
import concourse.mybir as mybir
import concourse.tile as tile
from concourse import bacc
from concourse.bass_utils import run_bass_kernel_spmd
from concourse.masks import make_identity

F32 = mybir.dt.float32
BF16 = mybir.dt.bfloat16
AF = mybir.ActivationFunctionType
OP = mybir.AluOpType

NC_ = 8          # cores
B, S, D, H, E, V = 2, 1024, 1024, 4096, 8, 32000
L, NH, HD = 2, 16, 64
T = B * S        # 2048 tokens
TS = T // NC_    # 256 tokens per core shard
VS = V // NC_    # 4000 vocab cols per core
HPC = NH // NC_  # 2 heads per core
KD = D // 128    # 8 feature chunks
KH = H // 128    # 32 hidden chunks
EPS = 1e-5

_COMPILED = None


# ---------------------------------------------------------------- device program

def _ln_tile(nc, pool, x_ap, g_ap, b_ap, out_ap, scale_ap=None):
    """LayerNorm over the free axis of a [128, W] fp32 tile.

    out = ((x - mean) * rstd * g + b) [* scale]  (scale: [128,1] per-token)
    """
    W = x_ap.shape[-1]
    mean = pool.tile([128, 1], F32, tag="ln_mean", bufs=4, name="ln_mean")
    nc.vector.reduce_sum(mean[:], x_ap, axis=mybir.AxisListType.X)
    nc.vector.tensor_scalar_mul(mean[:], mean[:], 1.0 / W)
    xc = pool.tile([128, W], F32, tag="ln_xc", bufs=1, name="ln_xc")
    nc.vector.tensor_scalar_sub(xc[:], x_ap, mean[:])
    sq = pool.tile([128, W], F32, tag="ln_sq", bufs=1, name="ln_sq")
    vs = pool.tile([128, 1], F32, tag="ln_vs", bufs=4, name="ln_vs")
    nc.scalar.activation(sq[:], xc[:], AF.Square, accum_out=vs[:])
    std = pool.tile([128, 1], F32, tag="ln_std", bufs=4, name="ln_std")
    nc.vector.tensor_scalar(std[:], vs[:], 1.0 / W, EPS, OP.mult, OP.add)
    nc.scalar.sqrt(std[:], std[:])
    rstd = pool.tile([128, 1], F32, tag="ln_rstd", bufs=4, name="ln_rstd")
    nc.vector.reciprocal(rstd[:], std[:])
    nc.vector.tensor_scalar_mul(xc[:], xc[:], rstd[:])
    nc.vector.tensor_tensor(xc[:], xc[:], g_ap, op=OP.mult)
    if scale_ap is None:
        nc.vector.tensor_tensor(out_ap, xc[:], b_ap, op=OP.add)
    else:
        nc.vector.tensor_tensor(xc[:], xc[:], b_ap, op=OP.add)
        nc.vector.tensor_scalar_mul(out_ap, xc[:], scale_ap)


def build_program(debug=False):
    nc = bacc.Bacc("TRN2", target_bir_lowering=False, debug=False,
                   enable_asserts=False, num_devices=NC_)

    # ---------------- I/O -------------------------------------------------
    x0Tb = nc.dram_tensor("x0Tb", [NC_, D, TS], BF16, kind="ExternalInput")
    x0_sh = nc.dram_tensor("x0_sh", [TS, D], F32, kind="ExternalInput")
    wqkvT = nc.dram_tensor("wqkvT", [L, D, 3 * 128], BF16, kind="ExternalInput")
    qkvb = nc.dram_tensor("qkvb", [L, 128, 3], F32, kind="ExternalInput")
    owT = nc.dram_tensor("owT", [L, 128, D], BF16, kind="ExternalInput")
    outb = nc.dram_tensor("outb", [L, 128, D], BF16, kind="ExternalInput")
    ln1g = nc.dram_tensor("ln1g", [L, 128, D], BF16, kind="ExternalInput")
    ln1b = nc.dram_tensor("ln1b", [L, 128, D], BF16, kind="ExternalInput")
    ln2g = nc.dram_tensor("ln2g", [L, 128, D], BF16, kind="ExternalInput")
    ln2b = nc.dram_tensor("ln2b", [L, 128, D], BF16, kind="ExternalInput")
    rwT32 = nc.dram_tensor("rwT32", [L, D, E], F32, kind="ExternalInput")
    rb = nc.dram_tensor("rb", [L, 128, E], F32, kind="ExternalInput")
    sel = nc.dram_tensor("sel", [128, E], F32, kind="ExternalInput")
    w1T = nc.dram_tensor("w1T", [L, D, H], BF16, kind="ExternalInput")
    b1 = nc.dram_tensor("b1", [L, 128, KH], F32, kind="ExternalInput")
    w2T = nc.dram_tensor("w2T", [L, H, D], BF16, kind="ExternalInput")
    b2r = nc.dram_tensor("b2r", [L, 128, D], BF16, kind="ExternalInput")
    egr = nc.dram_tensor("egr", [L, 128, D], BF16, kind="ExternalInput")
    ebr = nc.dram_tensor("ebr", [L, 128, D], BF16, kind="ExternalInput")
    projT = nc.dram_tensor("projT", [D, VS], BF16, kind="ExternalInput")
    pbr = nc.dram_tensor("pbr", [128, VS], F32, kind="ExternalInput")
    logits = nc.dram_tensor("logits", [T, VS], F32, kind="ExternalOutput")
    dbg = {}
    if debug:
        for l in range(L):
            dbg[f"rs1o_{l}"] = nc.dram_tensor(f"dbg_rs1o_{l}", [TS, D], F32, kind="ExternalOutput")
            dbg[f"agt_{l}"] = nc.dram_tensor(f"dbg_agt_{l}", [T, D], F32, kind="ExternalOutput")
            dbg[f"comb_{l}"] = nc.dram_tensor(f"dbg_comb_{l}", [T, E], F32, kind="ExternalOutput")
            dbg[f"rs2o_{l}"] = nc.dram_tensor(f"dbg_rs2o_{l}", [TS, D], F32, kind="ExternalOutput")
            dbg[f"qkv_{l}"] = nc.dram_tensor(f"dbg_qkv_{l}", [3 * 128, T], F32, kind="ExternalOutput")
            dbg[f"ctx_{l}"] = nc.dram_tensor(f"dbg_ctx_{l}", [128, T], F32, kind="ExternalOutput")
            dbg[f"xmid_{l}"] = nc.dram_tensor(f"dbg_xmid_{l}", [TS, D], F32, kind="ExternalOutput")
            dbg[f"tin_{l}"] = nc.dram_tensor(f"dbg_tin_{l}", [TS, D], F32, kind="ExternalOutput")
            dbg[f"agf_{l}"] = nc.dram_tensor(f"dbg_agf_{l}", [NC_ * D, TS], BF16, kind="ExternalOutput")

    with tile.TileContext(nc) as tc:
        es = ExitStack()       # SBUF pools released before the vocab phase
        esd = ExitStack()      # DRAM pool, kept open to the end
        pconst = es.enter_context(tc.tile_pool(name="pconst", bufs=1))
        pw = es.enter_context(tc.tile_pool(name="pw", bufs=1))
        pres = es.enter_context(tc.tile_pool(name="pres", bufs=1))
        playc = es.enter_context(tc.tile_pool(name="playc", bufs=1))
        pattw = es.enter_context(tc.tile_pool(name="pattw", bufs=1))
        pd = esd.enter_context(tc.tile_pool(name="pd", bufs=1, space="DRAM"))

        # constants
        ident_f = pconst.tile([128, 128], F32, name="ident_f")
        make_identity(nc, ident_f[:])
        ident_b = pconst.tile([128, 128], BF16, name="ident_b")
        make_identity(nc, ident_b[:])
        ones128 = pconst.tile([128, 1], BF16, name="ones128")
        nc.vector.memset(ones128[:], 1.0)
        ones64 = pconst.tile([1, 64], BF16, name="ones64")
        nc.vector.memset(ones64[:], 1.0)
        sel_sb = pconst.tile([128, E], F32, name="sel_sb")
        nc.sync.dma_start(sel_sb[:], sel[:])

        # residual-stream shard tiles (token-major [128, D] x 2 per generation)
        def new_xres(name):
            return [pres.tile([128, D], F32, tag="xres", bufs=3,
                              name=f"{name}_{mt}") for mt in range(2)]

        xres = new_xres("x0")
        for mt in range(2):
            nc.sync.dma_start(xres[mt][:], x0_sh[mt * 128:(mt + 1) * 128, :])

        # DRAM bounce buffers per layer
        def dram_bufs(l):
            return {
                "rs1_in": pd.tile([T, D], F32, name=f"rs1_in_{l}"),
                "rs1_out": pd.tile([TS, D], F32, name=f"rs1_out_{l}"),
                "agf_in": pd.tile([D, TS], BF16, name=f"agf_in_{l}"),
                "agf_out": pd.tile([NC_ * D, TS], BF16, addr_space="Shared", name=f"agf_out_{l}"),
                "agt_in": pd.tile([TS, D], F32, name=f"agt_in_{l}"),
                "agt_out": pd.tile([T, D], F32, addr_space="Shared", name=f"agt_out_{l}"),
                "rs2_in": pd.tile([T, D], F32, name=f"rs2_in_{l}"),
                "rs2_out": pd.tile([TS, D], F32, name=f"rs2_out_{l}"),
                "cmb_in": pd.tile([TS, E], F32, name=f"cmb_in_{l}"),
                "cmb_out": pd.tile([T, E], F32, addr_space="Shared", name=f"cmb_out_{l}"),
                "agx_in": pd.tile([D, TS], BF16, name=f"agx_in_{l}"),
                "agx_out": pd.tile([NC_ * D, TS], BF16, addr_space="Shared", name=f"agx_out_{l}"),
            }

        dbufs = [dram_bufs(l) for l in range(L)]
        rg = [list(range(NC_))]

        # x feature-major blocks for each stage: [NC_, D, TS] views
        xblocks = [x0Tb[:]]
        for l in range(L):
            xblocks.append(dbufs[l]["agx_out"].rearrange("(c p) t -> c p t", c=NC_))

        for l in range(L):
            lyr = ExitStack()
            pa = lyr.enter_context(tc.tile_pool(name=f"pa_{l}", bufs=1))

            # ---------------- attention ----------------------------------
            with nc.named_scope(f"l{l}_qkv"):
                wq = []
                for kc in range(KD):
                    t_ = pattw.tile([128, 3 * 128], BF16, tag=f"wqkv_{kc}", bufs=1,
                                    name=f"wqkv_{l}_{kc}")
                    nc.sync.dma_start(t_[:], wqkvT[l, kc * 128:(kc + 1) * 128, :])
                    wq.append(t_)
                qkvb_sb = pattw.tile([128, 3], F32, tag="qkvb", bufs=1, name=f"qkvb_{l}")
                nc.sync.dma_start(qkvb_sb[:], qkvb[l])
                ow_sb = pattw.tile([128, D], BF16, tag="ow", bufs=1, name=f"ow_{l}")
                nc.sync.dma_start(ow_sb[:], owT[l])

                qkv_sb = [pa.tile([128, T], BF16, tag=f"qkv{r}", bufs=1,
                                  name=f"qkv_{l}_{r}") for r in range(3)]
                with tc.tile_pool(name=f"pqp_{l}", bufs=1, space="PSUM") as pqp:
                    for st in range(4):  # 512-token supertiles
                        rhs_tiles = []
                        for kc in range(KD):
                            rhs = pa.tile([128, 512], BF16, tag=f"qr{kc}", bufs=2,
                                          name=f"qkvr_{l}_{st}_{kc}")
                            nc.sync.dma_start(
                                rhs.rearrange("p (b t) -> p b t", b=2),
                                xblocks[l][2 * st:2 * st + 2,
                                           kc * 128:(kc + 1) * 128, :].rearrange(
                                               "b p t -> p b t"))
                            rhs_tiles.append(rhs)
                        for r in range(3):
                            ps = pqp.tile([128, 512], F32, tag="qkv_ps", bufs=3,
                                          name=f"qkvps_{l}_{st}_{r}")
                            for kc in range(KD):
                                nc.tensor.matmul(
                                    ps[:], wq[kc][:, r * 128:(r + 1) * 128],
                                    rhs_tiles[kc][:],
                                    start=(kc == 0), stop=(kc == KD - 1))
                            nc.scalar.activation(
                                qkv_sb[r][:, st * 512:(st + 1) * 512], ps[:],
                                AF.Identity, bias=qkvb_sb[:, r:r + 1])

            with nc.named_scope(f"l{l}_attn"):
                ctxT = pa.tile([128, T], BF16, tag="ctxT", bufs=1, name=f"ctxT_{l}")
                with tc.tile_pool(name=f"ppair_{l}", bufs=1, space="PSUM") as ppr:
                    for b in range(B):
                        for hh in range(HPC):
                            qs = qkv_sb[0][hh * HD:(hh + 1) * HD, b * S:(b + 1) * S]
                            ks = qkv_sb[1][hh * HD:(hh + 1) * HD, b * S:(b + 1) * S]
                            vs_ = qkv_sb[2][hh * HD:(hh + 1) * HD, b * S:(b + 1) * S]
                            # v transposed to token-major
                            vtok = []
                            for kt in range(8):
                                pt = ppr.tile([128, 64], BF16, tag="vt_ps", bufs=1,
                                              name=f"vtp_{l}_{b}_{hh}_{kt}")
                                nc.tensor.transpose(
                                    pt[:], vs_[:, kt * 128:(kt + 1) * 128],
                                    ident_b[hh * HD:(hh + 1) * HD,
                                            hh * HD:(hh + 1) * HD])
                                vt = pa.tile([128, 64], BF16, tag=f"vtok{kt}", bufs=2,
                                             name=f"vtok_{l}_{b}_{hh}_{kt}")
                                nc.scalar.copy(vt[:], pt[:])
                                vtok.append(vt)
                            # scores (transposed [k, q]) -> exp
                            stx = [pa.tile([128, S], BF16, tag=f"st{kt}", bufs=2,
                                           name=f"st_{l}_{b}_{hh}_{kt}")
                                   for kt in range(8)]
                            for kt in range(8):
                                for qb in range(2):
                                    ps = ppr.tile([128, 512], F32, tag="s_ps", bufs=2,
                                                  name=f"sps_{l}_{b}_{hh}_{kt}_{qb}")
                                    nc.tensor.matmul(
                                        ps[:], ks[:, kt * 128:(kt + 1) * 128],
                                        qs[:, qb * 512:(qb + 1) * 512],
                                        start=True, stop=True)
                                    nc.scalar.activation(
                                        stx[kt][:, qb * 512:(qb + 1) * 512], ps[:],
                                        AF.Exp, scale=1.0 / math.sqrt(HD))
                            # softmax denominators via ones-matmul
                            sums = pa.tile([1, S], F32, tag="sums", bufs=1,
                                           name=f"sums_{l}_{b}_{hh}")
                            for qb in range(2):
                                ps = ppr.tile([1, 512], F32, tag="sum_ps", bufs=1,
                                              name=f"sumps_{l}_{b}_{hh}_{qb}")
                                for kt in range(8):
                                    nc.tensor.matmul(
                                        ps[:], ones128[:],
                                        stx[kt][:, qb * 512:(qb + 1) * 512],
                                        start=(kt == 0), stop=(kt == 7))
                                nc.scalar.copy(sums[:, qb * 512:(qb + 1) * 512], ps[:])
                            rec = pa.tile([1, S], F32, tag="rec", bufs=1,
                                          name=f"rec_{l}_{b}_{hh}")
                            nc.vector.reciprocal(rec[:], sums[:])
                            recb = pa.tile([1, S], BF16, tag="recb", bufs=1,
                                           name=f"recb_{l}_{b}_{hh}")
                            nc.vector.tensor_copy(recb[:], rec[:])
                            # ctx = v.T @ exp(sT), scaled by 1/sum (PE broadcast)
                            for qb in range(2):
                                pr = ppr.tile([64, 512], F32, tag="r_ps", bufs=1,
                                              name=f"rps_{l}_{b}_{hh}_{qb}")
                                nc.tensor.matmul(pr[:], ones64[:],
                                                 recb[:, qb * 512:(qb + 1) * 512],
                                                 start=True, stop=True)
                                rrep = pa.tile([64, 512], F32, tag="rrep", bufs=2,
                                               name=f"rrep_{l}_{b}_{hh}_{qb}")
                                nc.scalar.copy(rrep[:], pr[:])
                                pc = ppr.tile([64, 512], F32, tag="c_ps", bufs=2,
                                              name=f"cps_{l}_{b}_{hh}_{qb}")
                                for kt in range(8):
                                    nc.tensor.matmul(
                                        pc[:], vtok[kt][:],
                                        stx[kt][:, qb * 512:(qb + 1) * 512],
                                        start=(kt == 0), stop=(kt == 7))
                                nc.vector.tensor_tensor(
                                    ctxT[hh * HD:(hh + 1) * HD,
                                         b * S + qb * 512:b * S + (qb + 1) * 512],
                                    pc[:], rrep[:], op=OP.mult)

                # out projection partials (row-parallel over ctx features)
                with tc.tile_pool(name=f"pop_{l}", bufs=1, space="PSUM") as pop:
                    for mt in range(T // 128):
                        op_sb = pa.tile([128, D], F32, tag="oproj", bufs=2,
                                        name=f"oproj_{l}_{mt}")
                        for nb in range(2):
                            ps = pop.tile([128, 512], F32, tag="o_ps", bufs=3,
                                          name=f"ops_{l}_{mt}_{nb}")
                            nc.tensor.matmul(ps[:], ctxT[:, mt * 128:(mt + 1) * 128],
                                             ow_sb[:, nb * 512:(nb + 1) * 512],
                                             start=True, stop=True)
                            nc.scalar.copy(op_sb[:, nb * 512:(nb + 1) * 512], ps[:])
                        nc.sync.dma_start(
                            dbufs[l]["rs1_in"][mt * 128:(mt + 1) * 128, :], op_sb[:])

            if debug:
                for r in range(3):
                    qf = pa.tile([128, T], F32, tag="dbgq", bufs=1, name=f"dbgq_{l}_{r}")
                    nc.vector.tensor_copy(qf[:], qkv_sb[r][:])
                    nc.sync.dma_start(dbg[f"qkv_{l}"][r * 128:(r + 1) * 128, :], qf[:])
                cf = pa.tile([128, T], F32, tag="dbgq", bufs=1, name=f"dbgc_{l}")
                nc.vector.tensor_copy(cf[:], ctxT[:])
                nc.sync.dma_start(dbg[f"ctx_{l}"][:], cf[:])

            with nc.named_scope(f"l{l}_rs1"):
                nc.gpsimd.collective_compute(
                    "ReduceScatter", OP.add, replica_groups=rg,
                    ins=[dbufs[l]["rs1_in"][:]], outs=[dbufs[l]["rs1_out"][:]])

            lyr.close()

            # ---------------- LN1 + AGs + router --------------------------
            lyr = ExitStack()
            pb_ = lyr.enter_context(tc.tile_pool(name=f"pb_{l}", bufs=1))
            pbp = lyr.enter_context(tc.tile_pool(name=f"pbp_{l}", bufs=1, space="PSUM"))

            lc = {}
            for nm, src in [("ln1g", ln1g), ("ln1b", ln1b), ("ln2g", ln2g),
                            ("ln2b", ln2b), ("outb", outb), ("b2r", b2r),
                            ("egr", egr), ("ebr", ebr)]:
                t_ = playc.tile([128, D], BF16, tag=nm, bufs=1, name=f"{nm}_{l}")
                nc.sync.dma_start(t_[:], src[l])
                lc[nm] = t_

            xmid = [pres.tile([128, D], F32, tag="xmid", bufs=2,
                              name=f"xmid_{l}_{mt}") for mt in range(2)]
            rw32_sb = pattw.tile([128, KD, E], F32, tag="rw32", bufs=1,
                                 name=f"rw32_{l}")
            nc.sync.dma_start(rw32_sb[:],
                              rwT32[l].rearrange("(kc p) e -> p kc e", p=128))
            rb_sb = pattw.tile([128, E], F32, tag="rb", bufs=1, name=f"rb_{l}")
            nc.sync.dma_start(rb_sb[:], rb[l])
            with nc.named_scope(f"l{l}_ln1"):
                for mt in range(2):
                    tin = pb_.tile([128, D], F32, tag="rs1t", bufs=2,
                                   name=f"rs1t_{l}_{mt}")
                    nc.sync.dma_start(
                        tin[:], dbufs[l]["rs1_out"][mt * 128:(mt + 1) * 128, :])
                    nc.vector.tensor_tensor(tin[:], tin[:], xres[mt][:], op=OP.add)
                    nc.vector.tensor_tensor(tin[:], tin[:], lc["outb"][:], op=OP.add)
                    _ln_tile(nc, pb_, tin[:], lc["ln1g"][:], lc["ln1b"][:],
                             xmid[mt][:])
                    if debug:
                        nc.sync.dma_start(
                            dbg[f"tin_{l}"][mt * 128:(mt + 1) * 128, :], tin[:])
                        nc.sync.dma_start(
                            dbg[f"xmid_{l}"][mt * 128:(mt + 1) * 128, :], xmid[mt][:])
                    nc.sync.dma_start(
                        dbufs[l]["agt_in"][mt * 128:(mt + 1) * 128, :], xmid[mt][:])
                    xmT32 = pb_.tile([128, D], F32, tag="xmT32", bufs=2,
                                     name=f"xmT32_{l}_{mt}")
                    for kc in range(KD):
                        pt = pbp.tile([128, 128], F32, tag="tr_ps", bufs=2,
                                      name=f"trps_{l}_{mt}_{kc}")
                        nc.tensor.transpose(
                            pt[:], xmid[mt][:, kc * 128:(kc + 1) * 128], ident_f[:])
                        tb = pb_.tile([128, 128], BF16, tag="tr_sb", bufs=3,
                                      name=f"trsb_{l}_{mt}_{kc}")
                        nc.scalar.copy(tb[:], pt[:])
                        nc.vector.tensor_copy(xmT32[:, kc * 128:(kc + 1) * 128],
                                              pt[:])
                        nc.sync.dma_start(
                            dbufs[l]["agf_in"][kc * 128:(kc + 1) * 128,
                                               mt * 128:(mt + 1) * 128], tb[:])
                    # fp32 router logits for this 128-token tile
                    lg_ps = pbp.tile([128, E], F32, tag="lg_ps", bufs=2,
                                     name=f"lgps_{l}_{mt}")
                    for kc in range(KD):
                        nc.tensor.matmul(lg_ps[:],
                                         xmT32[:, kc * 128:(kc + 1) * 128],
                                         rw32_sb[:, kc, :],
                                         start=(kc == 0), stop=(kc == KD - 1))
                    lg = pb_.tile([128, E], F32, tag="lg", bufs=2,
                                  name=f"lg_{l}_{mt}")
                    nc.vector.tensor_tensor(lg[:], lg_ps[:], rb_sb[:], op=OP.add)
                    m1 = pb_.tile([128, 1], F32, tag="m1", bufs=2,
                                  name=f"m1_{l}_{mt}")
                    nc.vector.reduce_max(m1[:], lg[:], axis=mybir.AxisListType.X)
                    nc.vector.tensor_scalar_sub(lg[:], lg[:], m1[:])
                    ex = pb_.tile([128, E], F32, tag="ex", bufs=2,
                                  name=f"ex_{l}_{mt}")
                    nc.scalar.activation(ex[:], lg[:], AF.Exp)
                    gt = pb_.tile([128, E], F32, tag="gt", bufs=2,
                                  name=f"gt_{l}_{mt}")
                    nc.vector.tensor_scalar(gt[:], ex[:], 1.0, -2.0, OP.is_ge,
                                            OP.mult)
                    nc.vector.tensor_tensor(gt[:], ex[:], gt[:], op=OP.add)
                    m2 = pb_.tile([128, 1], F32, tag="m2", bufs=2,
                                  name=f"m2_{l}_{mt}")
                    nc.vector.reduce_max(m2[:], gt[:], axis=mybir.AxisListType.X)
                    keep = pb_.tile([128, E], F32, tag="keep", bufs=2,
                                    name=f"keep_{l}_{mt}")
                    nc.vector.tensor_scalar(keep[:], ex[:], m2[:], None, OP.is_ge)
                    den = pb_.tile([128, 1], F32, tag="den", bufs=2,
                                   name=f"den_{l}_{mt}")
                    nc.vector.tensor_scalar_add(den[:], m2[:], 1.0)
                    rden = pb_.tile([128, 1], F32, tag="rden", bufs=2,
                                    name=f"rden_{l}_{mt}")
                    nc.vector.reciprocal(rden[:], den[:])
                    nc.vector.tensor_tensor(keep[:], keep[:], ex[:], op=OP.mult)
                    nc.vector.tensor_scalar_mul(keep[:], keep[:], rden[:])
                    nc.sync.dma_start(
                        dbufs[l]["cmb_in"][mt * 128:(mt + 1) * 128, :], keep[:])

            with nc.named_scope(f"l{l}_ag"):
                nc.gpsimd.collective_compute(
                    "AllGather", OP.bypass, replica_groups=rg,
                    ins=[dbufs[l]["cmb_in"][:]], outs=[dbufs[l]["cmb_out"][:]])
                nc.gpsimd.collective_compute(
                    "AllGather", OP.bypass, replica_groups=rg,
                    ins=[dbufs[l]["agf_in"][:]], outs=[dbufs[l]["agf_out"][:]])
                nc.gpsimd.collective_compute(
                    "AllGather", OP.bypass, replica_groups=rg,
                    ins=[dbufs[l]["agt_in"][:]], outs=[dbufs[l]["agt_out"][:]])

            agf = dbufs[l]["agf_out"].rearrange("(c p) t -> c p t", c=NC_)

            lyr.close()

            # ---------------- MoE (dense, this core's expert) --------------
            lyr = ExitStack()
            pm = lyr.enter_context(tc.tile_pool(name=f"pm_{l}", bufs=1))
            pmp = lyr.enter_context(tc.tile_pool(name=f"pmp_{l}", bufs=1, space="PSUM"))

            w2_sb = []
            for kc in range(KH):
                t_ = pw.tile([128, D], BF16, tag=f"w2_{kc}", bufs=1,
                             name=f"w2_{l}_{kc}")
                nc.sync.dma_start(t_[:], w2T[l, kc * 128:(kc + 1) * 128, :])
                w2_sb.append(t_)
            b1_sb = pattw.tile([128, KH], F32, tag="b1", bufs=1, name=f"b1_{l}")
            nc.sync.dma_start(b1_sb[:], b1[l])

            with nc.named_scope(f"l{l}_moe"):
                for stt in range(4):  # 512-token supertiles
                    rhs_tiles = []
                    for kc in range(KD):
                        rhs = pm.tile([128, 512], BF16, tag=f"mrhs{kc}", bufs=2,
                                      name=f"mrhs_{l}_{stt}_{kc}")
                        nc.sync.dma_start(
                            rhs.rearrange("p (b t) -> p b t", b=2),
                            agf[2 * stt:2 * stt + 2,
                                kc * 128:(kc + 1) * 128, :].rearrange(
                                    "b p t -> p b t"))
                        rhs_tiles.append(rhs)
                    h_sb = [None] * KH
                    rev = stt % 2 == 1
                    hg_order = range(8)[::-1] if rev else range(8)
                    for hg in hg_order:  # stream fc1 weights in groups of 4 hc
                        w1g = []
                        for kc in range(KD):
                            wt = pm.tile([128, 512], BF16, tag=f"w1s{kc}", bufs=2,
                                         name=f"w1s_{l}_{stt}_{hg}_{kc}")
                            nc.sync.dma_start(
                                wt[:], w1T[l, kc * 128:(kc + 1) * 128,
                                           hg * 512:(hg + 1) * 512])
                            w1g.append(wt)
                        j_order = range(4)[::-1] if rev else range(4)
                        for j in j_order:
                            hc = hg * 4 + j
                            ps = pmp.tile([128, 512], F32, tag="f_ps", bufs=3,
                                          name=f"fps_{l}_{stt}_{hc}")
                            for kc in range(KD):
                                nc.tensor.matmul(
                                    ps[:], w1g[kc][:, j * 128:(j + 1) * 128],
                                    rhs_tiles[kc][:],
                                    start=(kc == 0), stop=(kc == KD - 1))
                            ht = pm.tile([128, 512], BF16, tag=f"h{hc}", bufs=1,
                                         name=f"h_{l}_{stt}_{hc}")
                            nc.scalar.activation(ht[:], ps[:], AF.Gelu,
                                                 bias=b1_sb[:, hc:hc + 1])
                            h_sb[hc] = ht
                    for mt in range(4):  # 128-token tiles within supertile
                        g = stt * 4 + mt
                        y = pm.tile([128, D], F32, tag="y", bufs=2,
                                    name=f"y_{l}_{g}")
                        for nb in range(2):
                            ps = pmp.tile([128, 512], F32, tag="g_ps", bufs=3,
                                          name=f"gps_{l}_{g}_{nb}")
                            for hc in range(KH):
                                nc.tensor.matmul(
                                    ps[:], h_sb[hc][:, mt * 128:(mt + 1) * 128],
                                    w2_sb[hc][:, nb * 512:(nb + 1) * 512],
                                    start=(hc == 0), stop=(hc == KH - 1))
                            nc.vector.tensor_tensor(
                                y[:, nb * 512:(nb + 1) * 512], ps[:],
                                lc["b2r"][:, nb * 512:(nb + 1) * 512], op=OP.add)
                        xm = pm.tile([128, D], F32, tag="xmtok", bufs=2,
                                     name=f"xmtok_{l}_{g}")
                        nc.sync.dma_start(
                            xm[:], dbufs[l]["agt_out"][g * 128:(g + 1) * 128, :])
                        nc.vector.tensor_tensor(y[:], y[:], xm[:], op=OP.add)
                        cmb8 = pm.tile([128, E], F32, tag="cmb8", bufs=2,
                                       name=f"cmb8_{l}_{g}")
                        nc.sync.dma_start(
                            cmb8[:], dbufs[l]["cmb_out"][g * 128:(g + 1) * 128, :])
                        nc.vector.tensor_tensor(cmb8[:], cmb8[:], sel_sb[:],
                                                op=OP.mult)
                        combg = pm.tile([128, 1], F32, tag="combg", bufs=2,
                                        name=f"combg_{l}_{g}")
                        nc.vector.reduce_sum(combg[:], cmb8[:],
                                             axis=mybir.AxisListType.X)
                        # expert LayerNorm + top-2 combine weight, in place on y
                        mean = pm.tile([128, 1], F32, tag="ln_mean", bufs=4,
                                       name=f"emean_{l}_{g}")
                        nc.vector.reduce_sum(mean[:], y[:],
                                             axis=mybir.AxisListType.X)
                        nc.vector.tensor_scalar_mul(mean[:], mean[:], 1.0 / D)
                        nc.vector.tensor_scalar_sub(y[:], y[:], mean[:])
                        sq = pm.tile([128, D], F32, tag="ln_sq", bufs=1,
                                     name=f"esq_{l}_{g}")
                        vs2 = pm.tile([128, 1], F32, tag="ln_vs", bufs=4,
                                      name=f"evs_{l}_{g}")
                        nc.scalar.activation(sq[:], y[:], AF.Square,
                                             accum_out=vs2[:])
                        std = pm.tile([128, 1], F32, tag="ln_std", bufs=4,
                                      name=f"estd_{l}_{g}")
                        nc.vector.tensor_scalar(std[:], vs2[:], 1.0 / D, EPS,
                                                OP.mult, OP.add)
                        nc.scalar.sqrt(std[:], std[:])
                        rstd = pm.tile([128, 1], F32, tag="ln_rstd", bufs=4,
                                       name=f"erstd_{l}_{g}")
                        nc.vector.reciprocal(rstd[:], std[:])
                        nc.vector.tensor_scalar_mul(y[:], y[:], rstd[:])
                        nc.vector.tensor_tensor(y[:], y[:], lc["egr"][:],
                                                op=OP.mult)
                        nc.vector.tensor_tensor(y[:], y[:], lc["ebr"][:],
                                                op=OP.add)
                        nc.vector.tensor_scalar_mul(y[:], y[:], combg[:])
                        nc.sync.dma_start(
                            dbufs[l]["rs2_in"][g * 128:(g + 1) * 128, :], y[:])

            with nc.named_scope(f"l{l}_rs2"):
                nc.gpsimd.collective_compute(
                    "ReduceScatter", OP.add, replica_groups=rg,
                    ins=[dbufs[l]["rs2_in"][:]], outs=[dbufs[l]["rs2_out"][:]])

            # ---------------- LN2 -> next x ------------------------------
            xout = new_xres(f"xout{l}")
            with nc.named_scope(f"l{l}_ln2"):
                for mt in range(2):
                    tin = pm.tile([128, D], F32, tag="rs2t", bufs=1,
                                  name=f"rs2t_{l}_{mt}")
                    nc.sync.dma_start(
                        tin[:], dbufs[l]["rs2_out"][mt * 128:(mt + 1) * 128, :])
                    nc.vector.tensor_tensor(tin[:], tin[:], xmid[mt][:], op=OP.add)
                    _ln_tile(nc, pm, tin[:], lc["ln2g"][:], lc["ln2b"][:],
                             xout[mt][:])
                    for kc in range(KD):
                        pt = pmp.tile([128, 128], F32, tag="tr2_ps", bufs=2,
                                      name=f"tr2ps_{l}_{mt}_{kc}")
                        nc.tensor.transpose(
                            pt[:], xout[mt][:, kc * 128:(kc + 1) * 128], ident_f[:])
                        tb = pm.tile([128, 128], BF16, tag="tr2_sb", bufs=3,
                                     name=f"tr2sb_{l}_{mt}_{kc}")
                        nc.scalar.copy(tb[:], pt[:])
                        nc.sync.dma_start(
                            dbufs[l]["agx_in"][kc * 128:(kc + 1) * 128,
                                               mt * 128:(mt + 1) * 128], tb[:])

            with nc.named_scope(f"l{l}_agx"):
                nc.gpsimd.collective_compute(
                    "AllGather", OP.bypass, replica_groups=rg,
                    ins=[dbufs[l]["agx_in"][:]], outs=[dbufs[l]["agx_out"][:]])

            if debug:
                nc.sync.dma_start(dbg[f"agf_{l}"][:], dbufs[l]["agf_out"][:])
                nc.sync.dma_start(dbg[f"rs1o_{l}"][:], dbufs[l]["rs1_out"][:])
                nc.sync.dma_start(dbg[f"agt_{l}"][:], dbufs[l]["agt_out"][:])
                nc.sync.dma_start(dbg[f"comb_{l}"][:], dbufs[l]["cmb_out"][:])
                nc.sync.dma_start(dbg[f"rs2o_{l}"][:], dbufs[l]["rs2_out"][:])

            xres = xout
            lyr.close()

        # ---------------- vocab projection (column-parallel) --------------
        es.close()  # release weight/activation pools so projT fits
        vx = ExitStack()
        pv = vx.enter_context(tc.tile_pool(name="pv", bufs=1))
        pvp = vx.enter_context(tc.tile_pool(name="pvp", bufs=1, space="PSUM"))
        with nc.named_scope("vocab"):
            pj_sb = []
            for kc in range(KD):
                t_ = pv.tile([128, VS], BF16, tag=f"pj{kc}", bufs=1,
                             name=f"pj_{kc}")
                nc.sync.dma_start(t_[:], projT[kc * 128:(kc + 1) * 128, :])
                pj_sb.append(t_)
            pb_sb = pv.tile([128, VS], F32, tag="pb", bufs=1, name="pb_sb")
            nc.sync.dma_start(pb_sb[:], pbr[:])
            agd = xblocks[L]
            for g in range(T // 128):
                blk, off = g // 2, (g % 2) * 128
                lhs_tiles = []
                for kc in range(KD):
                    xt = pv.tile([128, 128], BF16, tag=f"vxt{kc}", bufs=2,
                                 name=f"vxt_{g}_{kc}")
                    nc.sync.dma_start(
                        xt[:], agd[blk, kc * 128:(kc + 1) * 128, off:off + 128])
                    lhs_tiles.append(xt)
                for vb in range(8):
                    ps = pvp.tile([128, 500], F32, tag="v_ps", bufs=4,
                                  name=f"vps_{g}_{vb}")
                    for kc in range(KD):
                        nc.tensor.matmul(
                            ps[:], lhs_tiles[kc][:],
                            pj_sb[kc][:, vb * 500:(vb + 1) * 500],
                            start=(kc == 0), stop=(kc == KD - 1))
                    lo = pv.tile([128, 500], F32, tag="lo", bufs=4,
                                 name=f"lo_{g}_{vb}")
                    nc.vector.tensor_tensor(
                        lo[:], ps[:], pb_sb[:, vb * 500:(vb + 1) * 500], op=OP.add)
                    nc.sync.dma_start(
                        logits[g * 128:(g + 1) * 128, vb * 500:(vb + 1) * 500],
                        lo[:])
        vx.close()
        esd.close()

    nc.compile()
    return nc


# ---------------------------------------------------------------- host side

def _bf16(a):
    return np.ascontiguousarray(np.asarray(a).astype(ml_dtypes.bfloat16))


def _f32(a):
    return np.ascontiguousarray(np.asarray(a).astype(np.float32))


def _rep(v):
    """Replicate a [N] vector across 128 partitions -> [128, N]."""
    v = np.asarray(v, np.float32)
    return np.ascontiguousarray(np.broadcast_to(v, (128, v.shape[0])))


def _pos_encoding(S_, D_):
    pos = np.arange(S_, dtype=np.float32)[:, None]
    div = np.exp(np.arange(0, D_, 2, dtype=np.float32) * (-math.log(10000.0) / D_))
    pe = np.zeros((S_, D_), dtype=np.float32)
    pe[:, 0::2] = np.sin(pos * div)
    pe[:, 1::2] = np.cos(pos * div)
    return pe


def make_in_maps(input_ids, emb, qkv_w, qkv_b, out_w, out_b, ln1_g, ln1_b,
                 router_w, router_b, fc1_w, fc1_b, fc2_w, fc2_b, eln_g, eln_b,
                 ln2_g, ln2_b, proj_w, proj_b):
    input_ids = np.asarray(input_ids)
    emb = np.asarray(emb)
    qkv_w, qkv_b = np.asarray(qkv_w), np.asarray(qkv_b)
    out_w, out_b = np.asarray(out_w), np.asarray(out_b)
    ln1_g, ln1_b = np.asarray(ln1_g), np.asarray(ln1_b)
    router_w, router_b = np.asarray(router_w), np.asarray(router_b)
    fc1_w, fc1_b = np.asarray(fc1_w), np.asarray(fc1_b)
    fc2_w, fc2_b = np.asarray(fc2_w), np.asarray(fc2_b)
    eln_g, eln_b = np.asarray(eln_g), np.asarray(eln_b)
    ln2_g, ln2_b = np.asarray(ln2_g), np.asarray(ln2_b)
    proj_w, proj_b = np.asarray(proj_w), np.asarray(proj_b)

    ids = input_ids.reshape(T)
    pe = _pos_encoding(S, D)
    x0 = (emb[ids] * math.sqrt(D) + np.tile(pe, (B, 1))).astype(np.float32)
    x0Tb = _bf16(x0.reshape(NC_, TS, D).transpose(0, 2, 1))

    # shared (replicated) tensors
    shared = {
        "x0Tb": x0Tb,
        "rwT32": _f32(router_w.transpose(0, 2, 1)),
        "rb": _f32(np.stack([_rep(router_b[l]) for l in range(L)])),
        "outb": _f32(np.stack([_rep(out_b[l]) for l in range(L)])),
        "ln1g": _f32(np.stack([_rep(ln1_g[l]) for l in range(L)])),
        "ln1b": _f32(np.stack([_rep(ln1_b[l]) for l in range(L)])),
        "ln2g": _f32(np.stack([_rep(ln2_g[l]) for l in range(L)])),
        "ln2b": _f32(np.stack([_rep(ln2_b[l]) for l in range(L)])),
    }

    in_maps = []
    for c in range(NC_):
        m = dict(shared)
        m["x0_sh"] = _f32(x0[c * TS:(c + 1) * TS])
        wq = np.empty((L, D, 3 * 128), np.float32)
        qb = np.empty((L, 128, 3), np.float32)
        for l in range(L):
            for r in range(3):
                rows = slice(r * D + c * 128, r * D + (c + 1) * 128)
                wq[l, :, r * 128:(r + 1) * 128] = qkv_w[l, rows, :].T
                qb[l, :, r] = qkv_b[l, rows]
        m["wqkvT"] = _bf16(wq)
        m["qkvb"] = _f32(qb)
        m["owT"] = _bf16(out_w[:, :, c * 128:(c + 1) * 128].transpose(0, 2, 1))
        sel_ = np.zeros((128, E), np.float32)
        sel_[:, c] = 1.0
        m["sel"] = sel_
        m["w1T"] = _bf16(fc1_w[:, c].transpose(0, 2, 1))
        m["b1"] = _f32(fc1_b[:, c].reshape(L, KH, 128).transpose(0, 2, 1))
        m["w2T"] = _bf16(fc2_w[:, c].transpose(0, 2, 1))
        m["b2r"] = _f32(np.stack([_rep(fc2_b[l, c]) for l in range(L)]))
        m["egr"] = _f32(np.stack([_rep(eln_g[l, c]) for l in range(L)]))
        m["ebr"] = _f32(np.stack([_rep(eln_b[l, c]) for l in range(L)]))
        m["projT"] = _bf16(proj_w[c * VS:(c + 1) * VS].T)
        m["pbr"] = _f32(_rep(proj_b[c * VS:(c + 1) * VS]))
        in_maps.append(m)
    return in_maps


def get_compiled(debug=False):
    global _COMPILED
    if _COMPILED is None:
        _COMPILED = build_program(debug=debug)
    return _COMPILED


def kernel(_trace=False, _debug=False, **inputs):
    nc = get_compiled(debug=_debug)
    in_maps = make_in_maps(**inputs)
    res = run_bass_kernel_spmd(nc, in_maps, core_ids=list(range(NC_)),
                               trace=_trace)
    out = np.concatenate([res.results[c]["logits"] for c in range(NC_)], axis=1)
    out = out.reshape(B, S, V).astype(np.float32)
    kernel.last_exec_time_ns = res.exec_time_ns
    kernel.last_results = res.results
    return out


# revision 17
# speedup vs baseline: 1.0044x; 1.0044x over previous
"""MoE language model (2-layer transformer, top-2-of-8 MoE, 32k vocab projection)
distributed over 8 TRN2 NeuronCores.

Sharding:
  - attention: head-parallel (2 of 16 heads per core) + token-parallel epilogues
  - MoE: expert-parallel (1 expert per core), dense over tokens, combine via
    ReduceScatter of comb-weighted per-expert outputs
  - vocab projection: column-parallel (4000 of 32000 cols per core)
  - residual stream: token-sharded (256 tokens per core); AllGathers produce the
    replicated feature-major activations each matmul needs.

All matmuls run in bf16 (fp32 PSUM accumulation); everything else fp32.
"""

import math
from contextlib import ExitStack

import numpy as np
import ml_dtypes

import concourse.bass as bass
import concourse.mybir as mybir
import concourse.tile as tile
from concourse import bacc
from concourse.bass_utils import run_bass_kernel_spmd
from concourse.masks import make_identity

F32 = mybir.dt.float32
BF16 = mybir.dt.bfloat16
AF = mybir.ActivationFunctionType
OP = mybir.AluOpType

NC_ = 8          # cores
B, S, D, H, E, V = 2, 1024, 1024, 4096, 8, 32000
L, NH, HD = 2, 16, 64
T = B * S        # 2048 tokens
TS = T // NC_    # 256 tokens per core shard
VS = V // NC_    # 4000 vocab cols per core
HPC = NH // NC_  # 2 heads per core
KD = D // 128    # 8 feature chunks
KH = H // 128    # 32 hidden chunks
EPS = 1e-5

_COMPILED = None


# ---------------------------------------------------------------- device program

def _ln_tile(nc, pool, x_ap, g_ap, b_ap, out_ap, scale_ap=None):
    """LayerNorm over the free axis of a [128, W] fp32 tile.

    out = ((x - mean) * rstd * g + b) [* scale]  (scale: [128,1] per-token)
    """
    W = x_ap.shape[-1]
    mean = pool.tile([128, 1], F32, tag="ln_mean", bufs=4, name="ln_mean")
    nc.vector.reduce_sum(mean[:], x_ap, axis=mybir.AxisListType.X)
    nc.vector.tensor_scalar_mul(mean[:], mean[:], 1.0 / W)
    xc = pool.tile([128, W], F32, tag="ln_xc", bufs=1, name="ln_xc")
    nc.vector.tensor_scalar_sub(xc[:], x_ap, mean[:])
    sq = pool.tile([128, W], F32, tag="ln_sq", bufs=1, name="ln_sq")
    vs = pool.tile([128, 1], F32, tag="ln_vs", bufs=4, name="ln_vs")
    nc.scalar.activation(sq[:], xc[:], AF.Square, accum_out=vs[:])
    std = pool.tile([128, 1], F32, tag="ln_std", bufs=4, name="ln_std")
    nc.vector.tensor_scalar(std[:], vs[:], 1.0 / W, EPS, OP.mult, OP.add)
    nc.scalar.sqrt(std[:], std[:])
    rstd = pool.tile([128, 1], F32, tag="ln_rstd", bufs=4, name="ln_rstd")
    nc.vector.reciprocal(rstd[:], std[:])
    nc.vector.tensor_scalar_mul(xc[:], xc[:], rstd[:])
    nc.vector.tensor_tensor(xc[:], xc[:], g_ap, op=OP.mult)
    if scale_ap is None:
        nc.vector.tensor_tensor(out_ap, xc[:], b_ap, op=OP.add)
    else:
        nc.vector.tensor_tensor(xc[:], xc[:], b_ap, op=OP.add)
        nc.vector.tensor_scalar_mul(out_ap, xc[:], scale_ap)


def build_program(debug=False):
    nc = bacc.Bacc("TRN2", target_bir_lowering=False, debug=False,
                   enable_asserts=False, num_devices=NC_)

    # ---------------- I/O -------------------------------------------------
    x0Tb = nc.dram_tensor("x0Tb", [NC_, D, TS], BF16, kind="ExternalInput")
    x0_sh = nc.dram_tensor("x0_sh", [TS, D], F32, kind="ExternalInput")
    wqkvT = nc.dram_tensor("wqkvT", [L, D, 3 * 128], BF16, kind="ExternalInput")
    qkvb = nc.dram_tensor("qkvb", [L, 128, 3], F32, kind="ExternalInput")
    owT = nc.dram_tensor("owT", [L, 128, D], BF16, kind="ExternalInput")
    outb = nc.dram_tensor("outb", [L, 128, D], BF16, kind="ExternalInput")
    ln1g = nc.dram_tensor("ln1g", [L, 128, D], BF16, kind="ExternalInput")
    ln1b = nc.dram_tensor("ln1b", [L, 128, D], BF16, kind="ExternalInput")
    ln2g = nc.dram_tensor("ln2g", [L, 128, D], BF16, kind="ExternalInput")
    ln2b = nc.dram_tensor("ln2b", [L, 128, D], BF16, kind="ExternalInput")
    rwT32 = nc.dram_tensor("rwT32", [L, D, E], F32, kind="ExternalInput")
    rb = nc.dram_tensor("rb", [L, 128, E], F32, kind="ExternalInput")
    sel = nc.dram_tensor("sel", [128, E], F32, kind="ExternalInput")
    w1T = nc.dram_tensor("w1T", [L, D, H], BF16, kind="ExternalInput")
    b1 = nc.dram_tensor("b1", [L, 128, KH], F32, kind="ExternalInput")
    w2T = nc.dram_tensor("w2T", [L, H, D], BF16, kind="ExternalInput")
    b2r = nc.dram_tensor("b2r", [L, 128, D], BF16, kind="ExternalInput")
    egr = nc.dram_tensor("egr", [L, 128, D], BF16, kind="ExternalInput")
    ebr = nc.dram_tensor("ebr", [L, 128, D], BF16, kind="ExternalInput")
    projT = nc.dram_tensor("projT", [D, VS], BF16, kind="ExternalInput")
    pbr = nc.dram_tensor("pbr", [128, VS], F32, kind="ExternalInput")
    logits = nc.dram_tensor("logits", [T, VS], F32, kind="ExternalOutput")
    dbg = {}
    if debug:
        for l in range(L):
            dbg[f"rs1o_{l}"] = nc.dram_tensor(f"dbg_rs1o_{l}", [TS, D], F32, kind="ExternalOutput")
            dbg[f"agt_{l}"] = nc.dram_tensor(f"dbg_agt_{l}", [T, D], F32, kind="ExternalOutput")
            dbg[f"comb_{l}"] = nc.dram_tensor(f"dbg_comb_{l}", [T, E], F32, kind="ExternalOutput")
            dbg[f"rs2o_{l}"] = nc.dram_tensor(f"dbg_rs2o_{l}", [TS, D], F32, kind="ExternalOutput")
            dbg[f"qkv_{l}"] = nc.dram_tensor(f"dbg_qkv_{l}", [3 * 128, T], F32, kind="ExternalOutput")
            dbg[f"ctx_{l}"] = nc.dram_tensor(f"dbg_ctx_{l}", [128, T], F32, kind="ExternalOutput")
            dbg[f"xmid_{l}"] = nc.dram_tensor(f"dbg_xmid_{l}", [TS, D], F32, kind="ExternalOutput")
            dbg[f"tin_{l}"] = nc.dram_tensor(f"dbg_tin_{l}", [TS, D], F32, kind="ExternalOutput")
            dbg[f"agf_{l}"] = nc.dram_tensor(f"dbg_agf_{l}", [NC_ * D, TS], BF16, kind="ExternalOutput")

    with tile.TileContext(nc) as tc:
        es = ExitStack()       # SBUF pools released before the vocab phase
        esd = ExitStack()      # DRAM pool, kept open to the end
        pconst = es.enter_context(tc.tile_pool(name="pconst", bufs=1))
        pw = es.enter_context(tc.tile_pool(name="pw", bufs=1))
        pres = es.enter_context(tc.tile_pool(name="pres", bufs=1))
        playc = es.enter_context(tc.tile_pool(name="playc", bufs=1))
        pattw = es.enter_context(tc.tile_pool(name="pattw", bufs=1))
        pd = esd.enter_context(tc.tile_pool(name="pd", bufs=1, space="DRAM"))

        # constants
        ident_f = pconst.tile([128, 128], F32, name="ident_f")
        make_identity(nc, ident_f[:])
        ident_b = pconst.tile([128, 128], BF16, name="ident_b")
        make_identity(nc, ident_b[:])
        ones128 = pconst.tile([128, 1], BF16, name="ones128")
        nc.vector.memset(ones128[:], 1.0)
        ones64 = pconst.tile([1, 64], BF16, name="ones64")
        nc.vector.memset(ones64[:], 1.0)
        sel_sb = pconst.tile([128, E], F32, name="sel_sb")
        nc.sync.dma_start(sel_sb[:], sel[:])

        # residual-stream shard tiles (token-major [128, D] x 2 per generation)
        def new_xres(name):
            return [pres.tile([128, D], F32, tag="xres", bufs=3,
                              name=f"{name}_{mt}") for mt in range(2)]

        xres = new_xres("x0")
        for mt in range(2):
            nc.sync.dma_start(xres[mt][:], x0_sh[mt * 128:(mt + 1) * 128, :])

        # DRAM bounce buffers per layer
        def dram_bufs(l):
            return {
                "rs1_in": pd.tile([T, D], F32, name=f"rs1_in_{l}"),
                "rs1_out": pd.tile([TS, D], F32, name=f"rs1_out_{l}"),
                "agf_in": pd.tile([D, TS], BF16, name=f"agf_in_{l}"),
                "agf_out": pd.tile([NC_ * D, TS], BF16, addr_space="Shared", name=f"agf_out_{l}"),
                "agt_in": pd.tile([TS, D], F32, name=f"agt_in_{l}"),
                "agt_out": pd.tile([T, D], F32, addr_space="Shared", name=f"agt_out_{l}"),
                "rs2_in": pd.tile([T, D], F32, name=f"rs2_in_{l}"),
                "rs2_out": pd.tile([TS, D], F32, name=f"rs2_out_{l}"),
                "cmb_in": pd.tile([TS, E], F32, name=f"cmb_in_{l}"),
                "cmb_out": pd.tile([T, E], F32, addr_space="Shared", name=f"cmb_out_{l}"),
                "agx_in": pd.tile([D, TS], BF16, name=f"agx_in_{l}"),
                "agx_out": pd.tile([NC_ * D, TS], BF16, addr_space="Shared", name=f"agx_out_{l}"),
            }

        dbufs = [dram_bufs(l) for l in range(L)]
        rg = [list(range(NC_))]

        # x feature-major blocks for each stage: [NC_, D, TS] views
        xblocks = [x0Tb[:]]
        for l in range(L):
            xblocks.append(dbufs[l]["agx_out"].rearrange("(c p) t -> c p t", c=NC_))

        for l in range(L):
            lyr = ExitStack()
            pa = lyr.enter_context(tc.tile_pool(name=f"pa_{l}", bufs=1))

            # ---------------- attention ----------------------------------
            with nc.named_scope(f"l{l}_qkv"):
                wq = []
                for kc in range(KD):
                    t_ = pattw.tile([128, 3 * 128], BF16, tag=f"wqkv_{kc}", bufs=1,
                                    name=f"wqkv_{l}_{kc}")
                    nc.sync.dma_start(t_[:], wqkvT[l, kc * 128:(kc + 1) * 128, :])
                    wq.append(t_)
                qkvb_sb = pattw.tile([128, 3], F32, tag="qkvb", bufs=1, name=f"qkvb_{l}")
                nc.sync.dma_start(qkvb_sb[:], qkvb[l])
                ow_sb = pattw.tile([128, D], BF16, tag="ow", bufs=1, name=f"ow_{l}")
                nc.sync.dma_start(ow_sb[:], owT[l])

                qkv_sb = [pa.tile([128, T], BF16, tag=f"qkv{r}", bufs=1,
                                  name=f"qkv_{l}_{r}") for r in range(3)]
                with tc.tile_pool(name=f"pqp_{l}", bufs=1, space="PSUM") as pqp:
                    for st in range(4):  # 512-token supertiles
                        rhs_tiles = []
                        for kc in range(KD):
                            rhs = pa.tile([128, 512], BF16, tag=f"qr{kc}", bufs=2,
                                          name=f"qkvr_{l}_{st}_{kc}")
                            nc.sync.dma_start(
                                rhs.rearrange("p (b t) -> p b t", b=2),
                                xblocks[l][2 * st:2 * st + 2,
                                           kc * 128:(kc + 1) * 128, :].rearrange(
                                               "b p t -> p b t"))
                            rhs_tiles.append(rhs)
                        for r in range(3):
                            ps = pqp.tile([128, 512], F32, tag="qkv_ps", bufs=3,
                                          name=f"qkvps_{l}_{st}_{r}")
                            for kc in range(KD):
                                nc.tensor.matmul(
                                    ps[:], wq[kc][:, r * 128:(r + 1) * 128],
                                    rhs_tiles[kc][:],
                                    start=(kc == 0), stop=(kc == KD - 1))
                            nc.scalar.activation(
                                qkv_sb[r][:, st * 512:(st + 1) * 512], ps[:],
                                AF.Identity, bias=qkvb_sb[:, r:r + 1])

            with nc.named_scope(f"l{l}_attn"):
                ctxT = pa.tile([128, T], BF16, tag="ctxT", bufs=1, name=f"ctxT_{l}")
                with tc.tile_pool(name=f"ppair_{l}", bufs=1, space="PSUM") as ppr:
                    for b in range(B):
                        for hh in range(HPC):
                            qs = qkv_sb[0][hh * HD:(hh + 1) * HD, b * S:(b + 1) * S]
                            ks = qkv_sb[1][hh * HD:(hh + 1) * HD, b * S:(b + 1) * S]
                            vs_ = qkv_sb[2][hh * HD:(hh + 1) * HD, b * S:(b + 1) * S]
                            # v transposed to token-major
                            vtok = []
                            for kt in range(8):
                                pt = ppr.tile([128, 64], BF16, tag="vt_ps", bufs=1,
                                              name=f"vtp_{l}_{b}_{hh}_{kt}")
                                nc.tensor.transpose(
                                    pt[:], vs_[:, kt * 128:(kt + 1) * 128],
                                    ident_b[hh * HD:(hh + 1) * HD,
                                            hh * HD:(hh + 1) * HD])
                                vt = pa.tile([128, 64], BF16, tag=f"vtok{kt}", bufs=2,
                                             name=f"vtok_{l}_{b}_{hh}_{kt}")
                                nc.scalar.copy(vt[:], pt[:])
                                vtok.append(vt)
                            # scores (transposed [k, q]) -> exp
                            stx = [pa.tile([128, S], BF16, tag=f"st{kt}", bufs=2,
                                           name=f"st_{l}_{b}_{hh}_{kt}")
                                   for kt in range(8)]
                            for kt in range(8):
                                for qb in range(2):
                                    ps = ppr.tile([128, 512], F32, tag="s_ps", bufs=2,
                                                  name=f"sps_{l}_{b}_{hh}_{kt}_{qb}")
                                    nc.tensor.matmul(
                                        ps[:], ks[:, kt * 128:(kt + 1) * 128],
                                        qs[:, qb * 512:(qb + 1) * 512],
                                        start=True, stop=True)
                                    nc.scalar.activation(
                                        stx[kt][:, qb * 512:(qb + 1) * 512], ps[:],
                                        AF.Exp, scale=1.0 / math.sqrt(HD))
                            # softmax denominators via ones-matmul
                            sums = pa.tile([1, S], F32, tag="sums", bufs=1,
                                           name=f"sums_{l}_{b}_{hh}")
                            for qb in range(2):
                                ps = ppr.tile([1, 512], F32, tag="sum_ps", bufs=1,
                                              name=f"sumps_{l}_{b}_{hh}_{qb}")
                                for kt in range(8):
                                    nc.tensor.matmul(
                                        ps[:], ones128[:],
                                        stx[kt][:, qb * 512:(qb + 1) * 512],
                                        start=(kt == 0), stop=(kt == 7))
                                nc.scalar.copy(sums[:, qb * 512:(qb + 1) * 512], ps[:])
                            rec = pa.tile([1, S], F32, tag="rec", bufs=1,
                                          name=f"rec_{l}_{b}_{hh}")
                            nc.vector.reciprocal(rec[:], sums[:])
                            recb = pa.tile([1, S], BF16, tag="recb", bufs=1,
                                           name=f"recb_{l}_{b}_{hh}")
                            nc.vector.tensor_copy(recb[:], rec[:])
                            # ctx = v.T @ exp(sT), scaled by 1/sum (PE broadcast)
                            for qb in range(2):
                                pr = ppr.tile([64, 512], F32, tag="r_ps", bufs=1,
                                              name=f"rps_{l}_{b}_{hh}_{qb}")
                                nc.tensor.matmul(pr[:], ones64[:],
                                                 recb[:, qb * 512:(qb + 1) * 512],
                                                 start=True, stop=True)
                                rrep = pa.tile([64, 512], F32, tag="rrep", bufs=2,
                                               name=f"rrep_{l}_{b}_{hh}_{qb}")
                                nc.scalar.copy(rrep[:], pr[:])
                                pc = ppr.tile([64, 512], F32, tag="c_ps", bufs=2,
                                              name=f"cps_{l}_{b}_{hh}_{qb}")
                                for kt in range(8):
                                    nc.tensor.matmul(
                                        pc[:], vtok[kt][:],
                                        stx[kt][:, qb * 512:(qb + 1) * 512],
                                        start=(kt == 0), stop=(kt == 7))
                                nc.vector.tensor_tensor(
                                    ctxT[hh * HD:(hh + 1) * HD,
                                         b * S + qb * 512:b * S + (qb + 1) * 512],
                                    pc[:], rrep[:], op=OP.mult)

                # out projection partials (row-parallel over ctx features)
                with tc.tile_pool(name=f"pop_{l}", bufs=1, space="PSUM") as pop:
                    for mt in range(T // 128):
                        op_sb = pa.tile([128, D], F32, tag="oproj", bufs=2,
                                        name=f"oproj_{l}_{mt}")
                        for nb in range(2):
                            ps = pop.tile([128, 512], F32, tag="o_ps", bufs=3,
                                          name=f"ops_{l}_{mt}_{nb}")
                            nc.tensor.matmul(ps[:], ctxT[:, mt * 128:(mt + 1) * 128],
                                             ow_sb[:, nb * 512:(nb + 1) * 512],
                                             start=True, stop=True)
                            nc.scalar.copy(op_sb[:, nb * 512:(nb + 1) * 512], ps[:])
                        nc.sync.dma_start(
                            dbufs[l]["rs1_in"][mt * 128:(mt + 1) * 128, :], op_sb[:])

            if debug:
                for r in range(3):
                    qf = pa.tile([128, T], F32, tag="dbgq", bufs=1, name=f"dbgq_{l}_{r}")
                    nc.vector.tensor_copy(qf[:], qkv_sb[r][:])
                    nc.sync.dma_start(dbg[f"qkv_{l}"][r * 128:(r + 1) * 128, :], qf[:])
                cf = pa.tile([128, T], F32, tag="dbgq", bufs=1, name=f"dbgc_{l}")
                nc.vector.tensor_copy(cf[:], ctxT[:])
                nc.sync.dma_start(dbg[f"ctx_{l}"][:], cf[:])

            with nc.named_scope(f"l{l}_rs1"):
                nc.gpsimd.collective_compute(
                    "ReduceScatter", OP.add, replica_groups=rg,
                    ins=[dbufs[l]["rs1_in"][:]], outs=[dbufs[l]["rs1_out"][:]])

            lyr.close()

            # ---------------- LN1 + AGs + router --------------------------
            lyr = ExitStack()
            pb_ = lyr.enter_context(tc.tile_pool(name=f"pb_{l}", bufs=1))
            pbp = lyr.enter_context(tc.tile_pool(name=f"pbp_{l}", bufs=1, space="PSUM"))

            lc = {}
            for nm, src in [("ln1g", ln1g), ("ln1b", ln1b), ("ln2g", ln2g),
                            ("ln2b", ln2b), ("outb", outb), ("b2r", b2r),
                            ("egr", egr), ("ebr", ebr)]:
                t_ = playc.tile([128, D], BF16, tag=nm, bufs=1, name=f"{nm}_{l}")
                nc.sync.dma_start(t_[:], src[l])
                lc[nm] = t_

            xmid = [pres.tile([128, D], F32, tag="xmid", bufs=2,
                              name=f"xmid_{l}_{mt}") for mt in range(2)]
            rw32_sb = pattw.tile([128, KD, E], F32, tag="rw32", bufs=1,
                                 name=f"rw32_{l}")
            nc.sync.dma_start(rw32_sb[:],
                              rwT32[l].rearrange("(kc p) e -> p kc e", p=128))
            rb_sb = pattw.tile([128, E], F32, tag="rb", bufs=1, name=f"rb_{l}")
            nc.sync.dma_start(rb_sb[:], rb[l])
            with nc.named_scope(f"l{l}_ln1"):
                for mt in range(2):
                    tin = pb_.tile([128, D], F32, tag="rs1t", bufs=2,
                                   name=f"rs1t_{l}_{mt}")
                    nc.sync.dma_start(
                        tin[:], dbufs[l]["rs1_out"][mt * 128:(mt + 1) * 128, :])
                    nc.vector.tensor_tensor(tin[:], tin[:], xres[mt][:], op=OP.add)
                    nc.vector.tensor_tensor(tin[:], tin[:], lc["outb"][:], op=OP.add)
                    _ln_tile(nc, pb_, tin[:], lc["ln1g"][:], lc["ln1b"][:],
                             xmid[mt][:])
                    if debug:
                        nc.sync.dma_start(
                            dbg[f"tin_{l}"][mt * 128:(mt + 1) * 128, :], tin[:])
                        nc.sync.dma_start(
                            dbg[f"xmid_{l}"][mt * 128:(mt + 1) * 128, :], xmid[mt][:])
                    nc.sync.dma_start(
                        dbufs[l]["agt_in"][mt * 128:(mt + 1) * 128, :], xmid[mt][:])
                    xmT32 = pb_.tile([128, D], F32, tag="xmT32", bufs=2,
                                     name=f"xmT32_{l}_{mt}")
                    for kc in range(KD):
                        pt = pbp.tile([128, 128], F32, tag="tr_ps", bufs=2,
                                      name=f"trps_{l}_{mt}_{kc}")
                        nc.tensor.transpose(
                            pt[:], xmid[mt][:, kc * 128:(kc + 1) * 128], ident_f[:])
                        tb = pb_.tile([128, 128], BF16, tag="tr_sb", bufs=3,
                                      name=f"trsb_{l}_{mt}_{kc}")
                        nc.scalar.copy(tb[:], pt[:])
                        nc.vector.tensor_copy(xmT32[:, kc * 128:(kc + 1) * 128],
                                              pt[:])
                        nc.sync.dma_start(
                            dbufs[l]["agf_in"][kc * 128:(kc + 1) * 128,
                                               mt * 128:(mt + 1) * 128], tb[:])
                    # fp32 router logits for this 128-token tile
                    lg_ps = pbp.tile([128, E], F32, tag="lg_ps", bufs=2,
                                     name=f"lgps_{l}_{mt}")
                    for kc in range(KD):
                        nc.tensor.matmul(lg_ps[:],
                                         xmT32[:, kc * 128:(kc + 1) * 128],
                                         rw32_sb[:, kc, :],
                                         start=(kc == 0), stop=(kc == KD - 1))
                    lg = pb_.tile([128, E], F32, tag="lg", bufs=2,
                                  name=f"lg_{l}_{mt}")
                    nc.vector.tensor_tensor(lg[:], lg_ps[:], rb_sb[:], op=OP.add)
                    m1 = pb_.tile([128, 1], F32, tag="m1", bufs=2,
                                  name=f"m1_{l}_{mt}")
                    nc.vector.reduce_max(m1[:], lg[:], axis=mybir.AxisListType.X)
                    nc.vector.tensor_scalar_sub(lg[:], lg[:], m1[:])
                    ex = pb_.tile([128, E], F32, tag="ex", bufs=2,
                                  name=f"ex_{l}_{mt}")
                    nc.scalar.activation(ex[:], lg[:], AF.Exp)
                    gt = pb_.tile([128, E], F32, tag="gt", bufs=2,
                                  name=f"gt_{l}_{mt}")
                    nc.vector.tensor_scalar(gt[:], ex[:], 1.0, -2.0, OP.is_ge,
                                            OP.mult)
                    nc.vector.tensor_tensor(gt[:], ex[:], gt[:], op=OP.add)
                    m2 = pb_.tile([128, 1], F32, tag="m2", bufs=2,
                                  name=f"m2_{l}_{mt}")
                    nc.vector.reduce_max(m2[:], gt[:], axis=mybir.AxisListType.X)
                    keep = pb_.tile([128, E], F32, tag="keep", bufs=2,
                                    name=f"keep_{l}_{mt}")
                    nc.vector.tensor_scalar(keep[:], ex[:], m2[:], None, OP.is_ge)
                    den = pb_.tile([128, 1], F32, tag="den", bufs=2,
                                   name=f"den_{l}_{mt}")
                    nc.vector.tensor_scalar_add(den[:], m2[:], 1.0)
                    rden = pb_.tile([128, 1], F32, tag="rden", bufs=2,
                                    name=f"rden_{l}_{mt}")
                    nc.vector.reciprocal(rden[:], den[:])
                    nc.vector.tensor_tensor(keep[:], keep[:], ex[:], op=OP.mult)
                    nc.vector.tensor_scalar_mul(keep[:], keep[:], rden[:])
                    nc.sync.dma_start(
                        dbufs[l]["cmb_in"][mt * 128:(mt + 1) * 128, :], keep[:])

            with nc.named_scope(f"l{l}_ag"):
                nc.gpsimd.collective_compute(
                    "AllGather", OP.bypass, replica_groups=rg,
                    ins=[dbufs[l]["cmb_in"][:]], outs=[dbufs[l]["cmb_out"][:]])
                nc.gpsimd.collective_compute(
                    "AllGather", OP.bypass, replica_groups=rg,
                    ins=[dbufs[l]["agf_in"][:]], outs=[dbufs[l]["agf_out"][:]])
                nc.gpsimd.collective_compute(
                    "AllGather", OP.bypass, replica_groups=rg,
                    ins=[dbufs[l]["agt_in"][:]], outs=[dbufs[l]["agt_out"][:]])

            agf = dbufs[l]["agf_out"].rearrange("(c p) t -> c p t", c=NC_)

            lyr.close()

            # ---------------- MoE (dense, this core's expert) --------------
            lyr = ExitStack()
            pm = lyr.enter_context(tc.tile_pool(name=f"pm_{l}", bufs=1))
            pmp = lyr.enter_context(tc.tile_pool(name=f"pmp_{l}", bufs=1, space="PSUM"))

            w2_sb = []
            for kc in range(KH):
                t_ = pw.tile([128, D], BF16, tag=f"w2_{kc}", bufs=1,
                             name=f"w2_{l}_{kc}")
                nc.sync.dma_start(t_[:], w2T[l, kc * 128:(kc + 1) * 128, :])
                w2_sb.append(t_)
            b1_sb = pattw.tile([128, KH], F32, tag="b1", bufs=1, name=f"b1_{l}")
            nc.sync.dma_start(b1_sb[:], b1[l])

            with nc.named_scope(f"l{l}_moe"):
                for stt in range(4):  # 512-token supertiles
                    rhs_tiles = []
                    for kc in range(KD):
                        rhs = pm.tile([128, 512], BF16, tag=f"mrhs{kc}", bufs=2,
                                      name=f"mrhs_{l}_{stt}_{kc}")
                        nc.sync.dma_start(
                            rhs.rearrange("p (b t) -> p b t", b=2),
                            agf[2 * stt:2 * stt + 2,
                                kc * 128:(kc + 1) * 128, :].rearrange(
                                    "b p t -> p b t"))
                        rhs_tiles.append(rhs)
                    h_sb = [None] * KH
                    rev = stt % 2 == 1
                    hg_order = range(8)[::-1] if rev else range(8)
                    for hg in hg_order:  # stream fc1 weights in groups of 4 hc
                        w1g = []
                        for kc in range(KD):
                            wt = pm.tile([128, 512], BF16, tag=f"w1s{kc}", bufs=2,
                                         name=f"w1s_{l}_{stt}_{hg}_{kc}")
                            nc.sync.dma_start(
                                wt[:], w1T[l, kc * 128:(kc + 1) * 128,
                                           hg * 512:(hg + 1) * 512])
                            w1g.append(wt)
                        j_order = range(4)[::-1] if rev else range(4)
                        for j in j_order:
                            hc = hg * 4 + j
                            ps = pmp.tile([128, 512], F32, tag="f_ps", bufs=3,
                                          name=f"fps_{l}_{stt}_{hc}")
                            for kc in range(KD):
                                nc.tensor.matmul(
                                    ps[:], w1g[kc][:, j * 128:(j + 1) * 128],
                                    rhs_tiles[kc][:],
                                    start=(kc == 0), stop=(kc == KD - 1))
                            ht = pm.tile([128, 512], BF16, tag=f"h{hc}", bufs=1,
                                         name=f"h_{l}_{stt}_{hc}")
                            nc.scalar.activation(ht[:], ps[:], AF.Gelu,
                                                 bias=b1_sb[:, hc:hc + 1])
                            h_sb[hc] = ht
                    for mt in range(4):  # 128-token tiles within supertile
                        g = stt * 4 + mt
                        y = pm.tile([128, D], F32, tag="y", bufs=2,
                                    name=f"y_{l}_{g}")
                        for nb in range(2):
                            ps = pmp.tile([128, 512], F32, tag="g_ps", bufs=3,
                                          name=f"gps_{l}_{g}_{nb}")
                            for hc in range(KH):
                                nc.tensor.matmul(
                                    ps[:], h_sb[hc][:, mt * 128:(mt + 1) * 128],
                                    w2_sb[hc][:, nb * 512:(nb + 1) * 512],
                                    start=(hc == 0), stop=(hc == KH - 1))
                            nc.vector.tensor_tensor(
                                y[:, nb * 512:(nb + 1) * 512], ps[:],
                                lc["b2r"][:, nb * 512:(nb + 1) * 512], op=OP.add)
                        xm = pm.tile([128, D], F32, tag="xmtok", bufs=2,
                                     name=f"xmtok_{l}_{g}")
                        nc.sync.dma_start(
                            xm[:], dbufs[l]["agt_out"][g * 128:(g + 1) * 128, :])
                        nc.vector.tensor_tensor(y[:], y[:], xm[:], op=OP.add)
                        cmb8 = pm.tile([128, E], F32, tag="cmb8", bufs=2,
                                       name=f"cmb8_{l}_{g}")
                        nc.sync.dma_start(
                            cmb8[:], dbufs[l]["cmb_out"][g * 128:(g + 1) * 128, :])
                        nc.vector.tensor_tensor(cmb8[:], cmb8[:], sel_sb[:],
                                                op=OP.mult)
                        combg = pm.tile([128, 1], F32, tag="combg", bufs=2,
                                        name=f"combg_{l}_{g}")
                        nc.vector.reduce_sum(combg[:], cmb8[:],
                                             axis=mybir.AxisListType.X)
                        # expert LayerNorm + top-2 combine weight, in place on y
                        mean = pm.tile([128, 1], F32, tag="ln_mean", bufs=4,
                                       name=f"emean_{l}_{g}")
                        nc.vector.reduce_sum(mean[:], y[:],
                                             axis=mybir.AxisListType.X)
                        nc.vector.tensor_scalar_mul(mean[:], mean[:], 1.0 / D)
                        nc.vector.tensor_scalar_sub(y[:], y[:], mean[:])
                        sq = pm.tile([128, D], F32, tag="ln_sq", bufs=1,
                                     name=f"esq_{l}_{g}")
                        vs2 = pm.tile([128, 1], F32, tag="ln_vs", bufs=4,
                                      name=f"evs_{l}_{g}")
                        nc.scalar.activation(sq[:], y[:], AF.Square,
                                             accum_out=vs2[:])
                        std = pm.tile([128, 1], F32, tag="ln_std", bufs=4,
                                      name=f"estd_{l}_{g}")
                        nc.vector.tensor_scalar(std[:], vs2[:], 1.0 / D, EPS,
                                                OP.mult, OP.add)
                        nc.scalar.sqrt(std[:], std[:])
                        rstd = pm.tile([128, 1], F32, tag="ln_rstd", bufs=4,
                                       name=f"erstd_{l}_{g}")
                        nc.vector.reciprocal(rstd[:], std[:])
                        nc.vector.tensor_scalar_mul(y[:], y[:], rstd[:])
                        nc.vector.tensor_tensor(y[:], y[:], lc["egr"][:],
                                                op=OP.mult)
                        nc.vector.tensor_tensor(y[:], y[:], lc["ebr"][:],
                                                op=OP.add)
                        nc.vector.tensor_scalar_mul(y[:], y[:], combg[:])
                        nc.sync.dma_start(
                            dbufs[l]["rs2_in"][g * 128:(g + 1) * 128, :], y[:])

            with nc.named_scope(f"l{l}_rs2"):
                nc.gpsimd.collective_compute(
                    "ReduceScatter", OP.add, replica_groups=rg,
                    ins=[dbufs[l]["rs2_in"][:]], outs=[dbufs[l]["rs2_out"][:]])

            # ---------------- LN2 -> next x ------------------------------
            xout = new_xres(f"xout{l}")
            with nc.named_scope(f"l{l}_ln2"):
                for mt in range(2):
                    tin = pm.tile([128, D], F32, tag="rs2t", bufs=1,
                                  name=f"rs2t_{l}_{mt}")
                    nc.sync.dma_start(
                        tin[:], dbufs[l]["rs2_out"][mt * 128:(mt + 1) * 128, :])
                    nc.vector.tensor_tensor(tin[:], tin[:], xmid[mt][:], op=OP.add)
                    _ln_tile(nc, pm, tin[:], lc["ln2g"][:], lc["ln2b"][:],
                             xout[mt][:])
                    for kc in range(KD):
                        pt = pmp.tile([128, 128], F32, tag="tr2_ps", bufs=2,
                                      name=f"tr2ps_{l}_{mt}_{kc}")
                        nc.tensor.transpose(
                            pt[:], xout[mt][:, kc * 128:(kc + 1) * 128], ident_f[:])
                        tb = pm.tile([128, 128], BF16, tag="tr2_sb", bufs=3,
                                     name=f"tr2sb_{l}_{mt}_{kc}")
                        nc.scalar.copy(tb[:], pt[:])
                        nc.sync.dma_start(
                            dbufs[l]["agx_in"][kc * 128:(kc + 1) * 128,
                                               mt * 128:(mt + 1) * 128], tb[:])

            with nc.named_scope(f"l{l}_agx"):
                nc.gpsimd.collective_compute(
                    "AllGather", OP.bypass, replica_groups=rg,
                    ins=[dbufs[l]["agx_in"][:]], outs=[dbufs[l]["agx_out"][:]])

            if debug:
                nc.sync.dma_start(dbg[f"agf_{l}"][:], dbufs[l]["agf_out"][:])
                nc.sync.dma_start(dbg[f"rs1o_{l}"][:], dbufs[l]["rs1_out"][:])
                nc.sync.dma_start(dbg[f"agt_{l}"][:], dbufs[l]["agt_out"][:])
                nc.sync.dma_start(dbg[f"comb_{l}"][:], dbufs[l]["cmb_out"][:])
                nc.sync.dma_start(dbg[f"rs2o_{l}"][:], dbufs[l]["rs2_out"][:])

            xres = xout
            lyr.close()

        # ---------------- vocab projection (column-parallel) --------------
        es.close()  # release weight/activation pools so projT fits
        vx = ExitStack()
        pv = vx.enter_context(tc.tile_pool(name="pv", bufs=1))
        pvp = vx.enter_context(tc.tile_pool(name="pvp", bufs=1, space="PSUM"))
        with nc.named_scope("vocab"):
            pj_sb = []
            for kc in range(KD):
                t_ = pv.tile([128, VS], BF16, tag=f"pj{kc}", bufs=1,
                             name=f"pj_{kc}")
                nc.sync.dma_start(t_[:], projT[kc * 128:(kc + 1) * 128, :])
                pj_sb.append(t_)
            pb_sb = pv.tile([128, VS], F32, tag="pb", bufs=1, name="pb_sb")
            nc.sync.dma_start(pb_sb[:], pbr[:])
            agd = xblocks[L]
            for g in range(T // 128):
                blk, off = g // 2, (g % 2) * 128
                lhs_tiles = []
                for kc in range(KD):
                    xt = pv.tile([128, 128], BF16, tag=f"vxt{kc}", bufs=2,
                                 name=f"vxt_{g}_{kc}")
                    nc.sync.dma_start(
                        xt[:], agd[blk, kc * 128:(kc + 1) * 128, off:off + 128])
                    lhs_tiles.append(xt)
                for vb in range(8):
                    ps = pvp.tile([128, 500], F32, tag="v_ps", bufs=4,
                                  name=f"vps_{g}_{vb}")
                    for kc in range(KD):
                        nc.tensor.matmul(
                            ps[:], lhs_tiles[kc][:],
                            pj_sb[kc][:, vb * 500:(vb + 1) * 500],
                            start=(kc == 0), stop=(kc == KD - 1))
                    lo = pv.tile([128, 500], F32, tag="lo", bufs=4,
                                 name=f"lo_{g}_{vb}")
                    nc.vector.tensor_tensor(
                        lo[:], ps[:], pb_sb[:, vb * 500:(vb + 1) * 500], op=OP.add)
                    nc.sync.dma_start(
                        logits[g * 128:(g + 1) * 128, vb * 500:(vb + 1) * 500],
                        lo[:])
        vx.close()
        esd.close()

    nc.compile()
    return nc


# ---------------------------------------------------------------- host side

def _bf16(a):
    return np.ascontiguousarray(np.asarray(a).astype(ml_dtypes.bfloat16))


def _f32(a):
    return np.ascontiguousarray(np.asarray(a).astype(np.float32))


def _rep(v):
    """Replicate a [N] vector across 128 partitions -> [128, N]."""
    v = np.asarray(v, np.float32)
    return np.ascontiguousarray(np.broadcast_to(v, (128, v.shape[0])))


def _pos_encoding(S_, D_):
    pos = np.arange(S_, dtype=np.float32)[:, None]
    div = np.exp(np.arange(0, D_, 2, dtype=np.float32) * (-math.log(10000.0) / D_))
    pe = np.zeros((S_, D_), dtype=np.float32)
    pe[:, 0::2] = np.sin(pos * div)
    pe[:, 1::2] = np.cos(pos * div)
    return pe


def make_in_maps(input_ids, emb, qkv_w, qkv_b, out_w, out_b, ln1_g, ln1_b,
                 router_w, router_b, fc1_w, fc1_b, fc2_w, fc2_b, eln_g, eln_b,
                 ln2_g, ln2_b, proj_w, proj_b):
    input_ids = np.asarray(input_ids)
    emb = np.asarray(emb)
    qkv_w, qkv_b = np.asarray(qkv_w), np.asarray(qkv_b)
    out_w, out_b = np.asarray(out_w), np.asarray(out_b)
    ln1_g, ln1_b = np.asarray(ln1_g), np.asarray(ln1_b)
    router_w, router_b = np.asarray(router_w), np.asarray(router_b)
    fc1_w, fc1_b = np.asarray(fc1_w), np.asarray(fc1_b)
    fc2_w, fc2_b = np.asarray(fc2_w), np.asarray(fc2_b)
    eln_g, eln_b = np.asarray(eln_g), np.asarray(eln_b)
    ln2_g, ln2_b = np.asarray(ln2_g), np.asarray(ln2_b)
    proj_w, proj_b = np.asarray(proj_w), np.asarray(proj_b)

    ids = input_ids.reshape(T)
    pe = _pos_encoding(S, D)
    x0 = (emb[ids] * math.sqrt(D) + np.tile(pe, (B, 1))).astype(np.float32)
    x0Tb = _bf16(x0.reshape(NC_, TS, D).transpose(0, 2, 1))

    # shared (replicated) tensors
    shared = {
        "x0Tb": x0Tb,
        "rwT32": _f32(router_w.transpose(0, 2, 1)),
        "rb": _f32(np.stack([_rep(router_b[l]) for l in range(L)])),
        "outb": _f32(np.stack([_rep(out_b[l]) for l in range(L)])),
        "ln1g": _f32(np.stack([_rep(ln1_g[l]) for l in range(L)])),
        "ln1b": _f32(np.stack([_rep(ln1_b[l]) for l in range(L)])),
        "ln2g": _f32(np.stack([_rep(ln2_g[l]) for l in range(L)])),
        "ln2b": _f32(np.stack([_rep(ln2_b[l]) for l in range(L)])),
    }

    in_maps = []
    for c in range(NC_):
        m = dict(shared)
        m["x0_sh"] = _f32(x0[c * TS:(c + 1) * TS])
        wq = np.empty((L, D, 3 * 128), np.float32)
        qb = np.empty((L, 128, 3), np.float32)
        for l in range(L):
            for r in range(3):
                rows = slice(r * D + c * 128, r * D + (c + 1) * 128)
                wq[l, :, r * 128:(r + 1) * 128] = qkv_w[l, rows, :].T
                qb[l, :, r] = qkv_b[l, rows]
        m["wqkvT"] = _bf16(wq)
        m["qkvb"] = _f32(qb)
        m["owT"] = _bf16(out_w[:, :, c * 128:(c + 1) * 128].transpose(0, 2, 1))
        sel_ = np.zeros((128, E), np.float32)
        sel_[:, c] = 1.0
        m["sel"] = sel_
        m["w1T"] = _bf16(fc1_w[:, c].transpose(0, 2, 1))
        m["b1"] = _f32(fc1_b[:, c].reshape(L, KH, 128).transpose(0, 2, 1))
        m["w2T"] = _bf16(fc2_w[:, c].transpose(0, 2, 1))
        m["b2r"] = _f32(np.stack([_rep(fc2_b[l, c]) for l in range(L)]))
        m["egr"] = _f32(np.stack([_rep(eln_g[l, c]) for l in range(L)]))
        m["ebr"] = _f32(np.stack([_rep(eln_b[l, c]) for l in range(L)]))
        m["projT"] = _bf16(proj_w[c * VS:(c + 1) * VS].T)
        m["pbr"] = _f32(_rep(proj_b[c * VS:(c + 1) * VS]))
        in_maps.append(m)
    return in_maps


def get_compiled(debug=False):
    global _COMPILED
    if _COMPILED is None:
        _COMPILED = build_program(debug=debug)
    return _COMPILED


def kernel(_trace=False, _debug=False, **inputs):
    nc = get_compiled(debug=_debug)
    in_maps = make_in_maps(**inputs)
    res = run_bass_kernel_spmd(nc, in_maps, core_ids=list(range(NC_)),
                               trace=_trace)
    out = np.concatenate([res.results[c]["logits"] for c in range(NC_)], axis=1)
    out = out.reshape(B, S, V).astype(np.float32)
    kernel.last_exec_time_ns = res.exec_time_ns
    kernel.last_results = res.results
    kernel.last_scopes = res.per_core_scope_times
    return out


# revision 19
# speedup vs baseline: 1.0588x; 1.0541x over previous
"""MoE language model (2-layer transformer, top-2-of-8 MoE, 32k vocab projection)
distributed over 8 TRN2 NeuronCores.

Sharding:
  - attention: head-parallel (2 of 16 heads per core) + token-parallel epilogues
  - MoE: expert-parallel (1 expert per core), dense over tokens, combine via
    ReduceScatter of comb-weighted per-expert outputs
  - vocab projection: column-parallel (4000 of 32000 cols per core)
  - residual stream: token-sharded (256 tokens per core); AllGathers produce the
    replicated feature-major activations each matmul needs.

All matmuls run in bf16 (fp32 PSUM accumulation); everything else fp32.
"""

import math
from contextlib import ExitStack

import numpy as np
import ml_dtypes

import concourse.bass as bass
import concourse.mybir as mybir
import concourse.tile as tile
from concourse import bacc
from concourse.bass_utils import run_bass_kernel_spmd
from concourse.masks import make_identity

F32 = mybir.dt.float32
BF16 = mybir.dt.bfloat16
AF = mybir.ActivationFunctionType
OP = mybir.AluOpType

NC_ = 8          # cores
B, S, D, H, E, V = 2, 1024, 1024, 4096, 8, 32000
L, NH, HD = 2, 16, 64
T = B * S        # 2048 tokens
TS = T // NC_    # 256 tokens per core shard
VS = V // NC_    # 4000 vocab cols per core
HPC = NH // NC_  # 2 heads per core
KD = D // 128    # 8 feature chunks
KH = H // 128    # 32 hidden chunks
EPS = 1e-5

_COMPILED = None


# ---------------------------------------------------------------- device program

def _ln_tile(nc, pool, x_ap, g_ap, b_ap, out_ap, scale_ap=None):
    """LayerNorm over the free axis of a [128, W] fp32 tile.

    out = ((x - mean) * rstd * g + b) [* scale]  (scale: [128,1] per-token)
    """
    W = x_ap.shape[-1]
    mean = pool.tile([128, 1], F32, tag="ln_mean", bufs=4, name="ln_mean")
    nc.vector.reduce_sum(mean[:], x_ap, axis=mybir.AxisListType.X)
    nc.vector.tensor_scalar_mul(mean[:], mean[:], 1.0 / W)
    xc = pool.tile([128, W], F32, tag="ln_xc", bufs=1, name="ln_xc")
    nc.vector.tensor_scalar_sub(xc[:], x_ap, mean[:])
    sq = pool.tile([128, W], F32, tag="ln_sq", bufs=1, name="ln_sq")
    vs = pool.tile([128, 1], F32, tag="ln_vs", bufs=4, name="ln_vs")
    nc.scalar.activation(sq[:], xc[:], AF.Square, accum_out=vs[:])
    std = pool.tile([128, 1], F32, tag="ln_std", bufs=4, name="ln_std")
    nc.vector.tensor_scalar(std[:], vs[:], 1.0 / W, EPS, OP.mult, OP.add)
    nc.scalar.sqrt(std[:], std[:])
    rstd = pool.tile([128, 1], F32, tag="ln_rstd", bufs=4, name="ln_rstd")
    nc.vector.reciprocal(rstd[:], std[:])
    nc.vector.tensor_scalar_mul(xc[:], xc[:], rstd[:])
    nc.vector.tensor_tensor(xc[:], xc[:], g_ap, op=OP.mult)
    if scale_ap is None:
        nc.vector.tensor_tensor(out_ap, xc[:], b_ap, op=OP.add)
    else:
        nc.vector.tensor_tensor(xc[:], xc[:], b_ap, op=OP.add)
        nc.vector.tensor_scalar_mul(out_ap, xc[:], scale_ap)


def build_program(debug=False):
    nc = bacc.Bacc("TRN2", target_bir_lowering=False, debug=False,
                   enable_asserts=False, num_devices=NC_)

    # ---------------- I/O -------------------------------------------------
    x0Tb = nc.dram_tensor("x0Tb", [NC_, D, TS], BF16, kind="ExternalInput")
    x0_sh = nc.dram_tensor("x0_sh", [TS, D], F32, kind="ExternalInput")
    wqkvT = nc.dram_tensor("wqkvT", [L, D, 3 * 128], BF16, kind="ExternalInput")
    qkvb = nc.dram_tensor("qkvb", [L, 128, 3], F32, kind="ExternalInput")
    owT = nc.dram_tensor("owT", [L, 128, D], BF16, kind="ExternalInput")
    outb = nc.dram_tensor("outb", [L, 128, D], BF16, kind="ExternalInput")
    ln1g = nc.dram_tensor("ln1g", [L, 128, D], BF16, kind="ExternalInput")
    ln1b = nc.dram_tensor("ln1b", [L, 128, D], BF16, kind="ExternalInput")
    ln2g = nc.dram_tensor("ln2g", [L, 128, D], BF16, kind="ExternalInput")
    ln2b = nc.dram_tensor("ln2b", [L, 128, D], BF16, kind="ExternalInput")
    rwT32 = nc.dram_tensor("rwT32", [L, D, E], F32, kind="ExternalInput")
    rb = nc.dram_tensor("rb", [L, 128, E], F32, kind="ExternalInput")
    sel = nc.dram_tensor("sel", [128, E], F32, kind="ExternalInput")
    w1T = nc.dram_tensor("w1T", [L, D, H], BF16, kind="ExternalInput")
    b1 = nc.dram_tensor("b1", [L, 128, KH], F32, kind="ExternalInput")
    w2T = nc.dram_tensor("w2T", [L, H, D], BF16, kind="ExternalInput")
    b2r = nc.dram_tensor("b2r", [L, 128, D], BF16, kind="ExternalInput")
    egr = nc.dram_tensor("egr", [L, 128, D], BF16, kind="ExternalInput")
    ebr = nc.dram_tensor("ebr", [L, 128, D], BF16, kind="ExternalInput")
    projT = nc.dram_tensor("projT", [D, VS], BF16, kind="ExternalInput")
    pbr = nc.dram_tensor("pbr", [128, VS], F32, kind="ExternalInput")
    logits = nc.dram_tensor("logits", [T, VS], F32, kind="ExternalOutput")
    dbg = {}
    if debug:
        for l in range(L):
            dbg[f"rs1o_{l}"] = nc.dram_tensor(f"dbg_rs1o_{l}", [TS, D], F32, kind="ExternalOutput")
            dbg[f"agt_{l}"] = nc.dram_tensor(f"dbg_agt_{l}", [T, D], F32, kind="ExternalOutput")
            dbg[f"comb_{l}"] = nc.dram_tensor(f"dbg_comb_{l}", [T, E], F32, kind="ExternalOutput")
            dbg[f"rs2o_{l}"] = nc.dram_tensor(f"dbg_rs2o_{l}", [TS, D], F32, kind="ExternalOutput")
            dbg[f"qkv_{l}"] = nc.dram_tensor(f"dbg_qkv_{l}", [3 * 128, T], F32, kind="ExternalOutput")
            dbg[f"ctx_{l}"] = nc.dram_tensor(f"dbg_ctx_{l}", [128, T], F32, kind="ExternalOutput")
            dbg[f"xmid_{l}"] = nc.dram_tensor(f"dbg_xmid_{l}", [TS, D], F32, kind="ExternalOutput")
            dbg[f"tin_{l}"] = nc.dram_tensor(f"dbg_tin_{l}", [TS, D], F32, kind="ExternalOutput")
            dbg[f"agf_{l}"] = nc.dram_tensor(f"dbg_agf_{l}", [NC_ * D, TS], BF16, kind="ExternalOutput")

    with tile.TileContext(nc) as tc:
        es = ExitStack()       # SBUF pools released before the vocab phase
        esd = ExitStack()      # DRAM pool, kept open to the end
        pconst = es.enter_context(tc.tile_pool(name="pconst", bufs=1))
        pw = es.enter_context(tc.tile_pool(name="pw", bufs=1))
        pres = es.enter_context(tc.tile_pool(name="pres", bufs=1))
        playc = es.enter_context(tc.tile_pool(name="playc", bufs=1))
        pattw = es.enter_context(tc.tile_pool(name="pattw", bufs=1))
        pd = esd.enter_context(tc.tile_pool(name="pd", bufs=1, space="DRAM"))

        # constants
        ident_f = pconst.tile([128, 128], F32, name="ident_f")
        make_identity(nc, ident_f[:])
        ident_b = pconst.tile([128, 128], BF16, name="ident_b")
        make_identity(nc, ident_b[:])
        ones128 = pconst.tile([128, 1], BF16, name="ones128")
        nc.vector.memset(ones128[:], 1.0)
        ones64 = pconst.tile([1, 64], BF16, name="ones64")
        nc.vector.memset(ones64[:], 1.0)
        sel_sb = pconst.tile([128, E], F32, name="sel_sb")
        nc.sync.dma_start(sel_sb[:], sel[:])

        # residual-stream shard tiles (token-major [128, D] x 2 per generation)
        def new_xres(name):
            return [pres.tile([128, D], F32, tag="xres", bufs=3,
                              name=f"{name}_{mt}") for mt in range(2)]

        xres = new_xres("x0")
        for mt in range(2):
            nc.sync.dma_start(xres[mt][:], x0_sh[mt * 128:(mt + 1) * 128, :])

        # DRAM bounce buffers per layer
        def dram_bufs(l):
            return {
                "rs1_in": pd.tile([T, D], BF16, name=f"rs1_in_{l}"),
                "rs1_out": pd.tile([TS, D], BF16, name=f"rs1_out_{l}"),
                "agf_in": pd.tile([D, TS], BF16, name=f"agf_in_{l}"),
                "agf_out": pd.tile([NC_ * D, TS], BF16, addr_space="Shared", name=f"agf_out_{l}"),
                "agt_in": pd.tile([TS, D], F32, name=f"agt_in_{l}"),
                "agt_out": pd.tile([T, D], F32, addr_space="Shared", name=f"agt_out_{l}"),
                "rs2_in": pd.tile([T, D], BF16, name=f"rs2_in_{l}"),
                "rs2_out": pd.tile([TS, D], BF16, name=f"rs2_out_{l}"),
                "cmb_in": pd.tile([TS, E], F32, name=f"cmb_in_{l}"),
                "cmb_out": pd.tile([T, E], F32, addr_space="Shared", name=f"cmb_out_{l}"),
                "agx_in": pd.tile([D, TS], BF16, name=f"agx_in_{l}"),
                "agx_out": pd.tile([NC_ * D, TS], BF16, addr_space="Shared", name=f"agx_out_{l}"),
            }

        dbufs = [dram_bufs(l) for l in range(L)]
        rg = [list(range(NC_))]

        # x feature-major blocks for each stage: [NC_, D, TS] views
        xblocks = [x0Tb[:]]
        for l in range(L):
            xblocks.append(dbufs[l]["agx_out"].rearrange("(c p) t -> c p t", c=NC_))

        for l in range(L):
            lyr = ExitStack()
            pa = lyr.enter_context(tc.tile_pool(name=f"pa_{l}", bufs=1))

            # ---------------- attention ----------------------------------
            with nc.named_scope(f"l{l}_qkv"):
                wq = []
                for kc in range(KD):
                    t_ = pattw.tile([128, 3 * 128], BF16, tag=f"wqkv_{kc}", bufs=1,
                                    name=f"wqkv_{l}_{kc}")
                    nc.sync.dma_start(t_[:], wqkvT[l, kc * 128:(kc + 1) * 128, :])
                    wq.append(t_)
                qkvb_sb = pattw.tile([128, 3], F32, tag="qkvb", bufs=1, name=f"qkvb_{l}")
                nc.sync.dma_start(qkvb_sb[:], qkvb[l])
                ow_sb = pattw.tile([128, D], BF16, tag="ow", bufs=1, name=f"ow_{l}")
                nc.sync.dma_start(ow_sb[:], owT[l])

                qkv_sb = [pa.tile([128, T], BF16, tag=f"qkv{r}", bufs=1,
                                  name=f"qkv_{l}_{r}") for r in range(3)]
                with tc.tile_pool(name=f"pqp_{l}", bufs=1, space="PSUM") as pqp:
                    for st in range(4):  # 512-token supertiles
                        rhs_tiles = []
                        for kc in range(KD):
                            rhs = pa.tile([128, 512], BF16, tag=f"qr{kc}", bufs=2,
                                          name=f"qkvr_{l}_{st}_{kc}")
                            nc.sync.dma_start(
                                rhs.rearrange("p (b t) -> p b t", b=2),
                                xblocks[l][2 * st:2 * st + 2,
                                           kc * 128:(kc + 1) * 128, :].rearrange(
                                               "b p t -> p b t"))
                            rhs_tiles.append(rhs)
                        for r in range(3):
                            ps = pqp.tile([128, 512], F32, tag="qkv_ps", bufs=3,
                                          name=f"qkvps_{l}_{st}_{r}")
                            for kc in range(KD):
                                nc.tensor.matmul(
                                    ps[:], wq[kc][:, r * 128:(r + 1) * 128],
                                    rhs_tiles[kc][:],
                                    start=(kc == 0), stop=(kc == KD - 1))
                            nc.scalar.activation(
                                qkv_sb[r][:, st * 512:(st + 1) * 512], ps[:],
                                AF.Identity, bias=qkvb_sb[:, r:r + 1])

            with nc.named_scope(f"l{l}_attn"):
                ctxT = pa.tile([128, T], BF16, tag="ctxT", bufs=1, name=f"ctxT_{l}")
                with tc.tile_pool(name=f"ppair_{l}", bufs=1, space="PSUM") as ppr:
                    for b in range(B):
                        for hh in range(HPC):
                            qs = qkv_sb[0][hh * HD:(hh + 1) * HD, b * S:(b + 1) * S]
                            ks = qkv_sb[1][hh * HD:(hh + 1) * HD, b * S:(b + 1) * S]
                            vs_ = qkv_sb[2][hh * HD:(hh + 1) * HD, b * S:(b + 1) * S]
                            # v transposed to token-major
                            vtok = []
                            for kt in range(8):
                                pt = ppr.tile([128, 64], BF16, tag="vt_ps", bufs=1,
                                              name=f"vtp_{l}_{b}_{hh}_{kt}")
                                nc.tensor.transpose(
                                    pt[:], vs_[:, kt * 128:(kt + 1) * 128],
                                    ident_b[hh * HD:(hh + 1) * HD,
                                            hh * HD:(hh + 1) * HD])
                                vt = pa.tile([128, 64], BF16, tag=f"vtok{kt}", bufs=2,
                                             name=f"vtok_{l}_{b}_{hh}_{kt}")
                                nc.scalar.copy(vt[:], pt[:])
                                vtok.append(vt)
                            # scores (transposed [k, q]) -> exp
                            stx = [pa.tile([128, S], BF16, tag=f"st{kt}", bufs=2,
                                           name=f"st_{l}_{b}_{hh}_{kt}")
                                   for kt in range(8)]
                            for kt in range(8):
                                for qb in range(2):
                                    ps = ppr.tile([128, 512], F32, tag="s_ps", bufs=2,
                                                  name=f"sps_{l}_{b}_{hh}_{kt}_{qb}")
                                    nc.tensor.matmul(
                                        ps[:], ks[:, kt * 128:(kt + 1) * 128],
                                        qs[:, qb * 512:(qb + 1) * 512],
                                        start=True, stop=True)
                                    nc.scalar.activation(
                                        stx[kt][:, qb * 512:(qb + 1) * 512], ps[:],
                                        AF.Exp, scale=1.0 / math.sqrt(HD))
                            # softmax denominators via ones-matmul
                            sums = pa.tile([1, S], F32, tag="sums", bufs=1,
                                           name=f"sums_{l}_{b}_{hh}")
                            for qb in range(2):
                                ps = ppr.tile([1, 512], F32, tag="sum_ps", bufs=1,
                                              name=f"sumps_{l}_{b}_{hh}_{qb}")
                                for kt in range(8):
                                    nc.tensor.matmul(
                                        ps[:], ones128[:],
                                        stx[kt][:, qb * 512:(qb + 1) * 512],
                                        start=(kt == 0), stop=(kt == 7))
                                nc.scalar.copy(sums[:, qb * 512:(qb + 1) * 512], ps[:])
                            rec = pa.tile([1, S], F32, tag="rec", bufs=1,
                                          name=f"rec_{l}_{b}_{hh}")
                            nc.vector.reciprocal(rec[:], sums[:])
                            recb = pa.tile([1, S], BF16, tag="recb", bufs=1,
                                           name=f"recb_{l}_{b}_{hh}")
                            nc.vector.tensor_copy(recb[:], rec[:])
                            # ctx = v.T @ exp(sT), scaled by 1/sum (PE broadcast)
                            for qb in range(2):
                                pr = ppr.tile([64, 512], F32, tag="r_ps", bufs=1,
                                              name=f"rps_{l}_{b}_{hh}_{qb}")
                                nc.tensor.matmul(pr[:], ones64[:],
                                                 recb[:, qb * 512:(qb + 1) * 512],
                                                 start=True, stop=True)
                                rrep = pa.tile([64, 512], F32, tag="rrep", bufs=2,
                                               name=f"rrep_{l}_{b}_{hh}_{qb}")
                                nc.scalar.copy(rrep[:], pr[:])
                                pc = ppr.tile([64, 512], F32, tag="c_ps", bufs=2,
                                              name=f"cps_{l}_{b}_{hh}_{qb}")
                                for kt in range(8):
                                    nc.tensor.matmul(
                                        pc[:], vtok[kt][:],
                                        stx[kt][:, qb * 512:(qb + 1) * 512],
                                        start=(kt == 0), stop=(kt == 7))
                                nc.vector.tensor_tensor(
                                    ctxT[hh * HD:(hh + 1) * HD,
                                         b * S + qb * 512:b * S + (qb + 1) * 512],
                                    pc[:], rrep[:], op=OP.mult)

                # out projection partials (row-parallel over ctx features)
                with tc.tile_pool(name=f"pop_{l}", bufs=1, space="PSUM") as pop:
                    for mt in range(T // 128):
                        op_sb = pa.tile([128, D], BF16, tag="oproj", bufs=2,
                                        name=f"oproj_{l}_{mt}")
                        for nb in range(2):
                            ps = pop.tile([128, 512], F32, tag="o_ps", bufs=3,
                                          name=f"ops_{l}_{mt}_{nb}")
                            nc.tensor.matmul(ps[:], ctxT[:, mt * 128:(mt + 1) * 128],
                                             ow_sb[:, nb * 512:(nb + 1) * 512],
                                             start=True, stop=True)
                            nc.scalar.copy(op_sb[:, nb * 512:(nb + 1) * 512], ps[:])
                        nc.sync.dma_start(
                            dbufs[l]["rs1_in"][mt * 128:(mt + 1) * 128, :], op_sb[:])

            if debug:
                for r in range(3):
                    qf = pa.tile([128, T], F32, tag="dbgq", bufs=1, name=f"dbgq_{l}_{r}")
                    nc.vector.tensor_copy(qf[:], qkv_sb[r][:])
                    nc.sync.dma_start(dbg[f"qkv_{l}"][r * 128:(r + 1) * 128, :], qf[:])
                cf = pa.tile([128, T], F32, tag="dbgq", bufs=1, name=f"dbgc_{l}")
                nc.vector.tensor_copy(cf[:], ctxT[:])
                nc.sync.dma_start(dbg[f"ctx_{l}"][:], cf[:])

            with nc.named_scope(f"l{l}_rs1"):
                nc.gpsimd.collective_compute(
                    "ReduceScatter", OP.add, replica_groups=rg,
                    ins=[dbufs[l]["rs1_in"][:]], outs=[dbufs[l]["rs1_out"][:]])

            lyr.close()

            # ---------------- LN1 + AGs + router --------------------------
            lyr = ExitStack()
            pb_ = lyr.enter_context(tc.tile_pool(name=f"pb_{l}", bufs=1))
            pbp = lyr.enter_context(tc.tile_pool(name=f"pbp_{l}", bufs=1, space="PSUM"))

            lc = {}
            for nm, src in [("ln1g", ln1g), ("ln1b", ln1b), ("ln2g", ln2g),
                            ("ln2b", ln2b), ("outb", outb), ("b2r", b2r),
                            ("egr", egr), ("ebr", ebr)]:
                t_ = playc.tile([128, D], BF16, tag=nm, bufs=1, name=f"{nm}_{l}")
                nc.sync.dma_start(t_[:], src[l])
                lc[nm] = t_

            xmid = [pres.tile([128, D], F32, tag="xmid", bufs=2,
                              name=f"xmid_{l}_{mt}") for mt in range(2)]
            rw32_sb = pattw.tile([128, KD, E], F32, tag="rw32", bufs=1,
                                 name=f"rw32_{l}")
            nc.sync.dma_start(rw32_sb[:],
                              rwT32[l].rearrange("(kc p) e -> p kc e", p=128))
            rb_sb = pattw.tile([128, E], F32, tag="rb", bufs=1, name=f"rb_{l}")
            nc.sync.dma_start(rb_sb[:], rb[l])
            with nc.named_scope(f"l{l}_ln1"):
                for mt in range(2):
                    tinb = pb_.tile([128, D], BF16, tag="rs1tb", bufs=2,
                                    name=f"rs1tb_{l}_{mt}")
                    nc.sync.dma_start(
                        tinb[:], dbufs[l]["rs1_out"][mt * 128:(mt + 1) * 128, :])
                    tin = pb_.tile([128, D], F32, tag="rs1t", bufs=2,
                                   name=f"rs1t_{l}_{mt}")
                    nc.vector.tensor_tensor(tin[:], tinb[:], xres[mt][:], op=OP.add)
                    nc.vector.tensor_tensor(tin[:], tin[:], lc["outb"][:], op=OP.add)
                    _ln_tile(nc, pb_, tin[:], lc["ln1g"][:], lc["ln1b"][:],
                             xmid[mt][:])
                    if debug:
                        nc.sync.dma_start(
                            dbg[f"tin_{l}"][mt * 128:(mt + 1) * 128, :], tin[:])
                        nc.sync.dma_start(
                            dbg[f"xmid_{l}"][mt * 128:(mt + 1) * 128, :], xmid[mt][:])
                    nc.sync.dma_start(
                        dbufs[l]["agt_in"][mt * 128:(mt + 1) * 128, :], xmid[mt][:])
                    xmT32 = pb_.tile([128, D], F32, tag="xmT32", bufs=2,
                                     name=f"xmT32_{l}_{mt}")
                    for kc in range(KD):
                        pt = pbp.tile([128, 128], F32, tag="tr_ps", bufs=2,
                                      name=f"trps_{l}_{mt}_{kc}")
                        nc.tensor.transpose(
                            pt[:], xmid[mt][:, kc * 128:(kc + 1) * 128], ident_f[:])
                        tb = pb_.tile([128, 128], BF16, tag="tr_sb", bufs=3,
                                      name=f"trsb_{l}_{mt}_{kc}")
                        nc.scalar.copy(tb[:], pt[:])
                        nc.vector.tensor_copy(xmT32[:, kc * 128:(kc + 1) * 128],
                                              pt[:])
                        nc.sync.dma_start(
                            dbufs[l]["agf_in"][kc * 128:(kc + 1) * 128,
                                               mt * 128:(mt + 1) * 128], tb[:])
                    # fp32 router logits for this 128-token tile
                    lg_ps = pbp.tile([128, E], F32, tag="lg_ps", bufs=2,
                                     name=f"lgps_{l}_{mt}")
                    for kc in range(KD):
                        nc.tensor.matmul(lg_ps[:],
                                         xmT32[:, kc * 128:(kc + 1) * 128],
                                         rw32_sb[:, kc, :],
                                         start=(kc == 0), stop=(kc == KD - 1))
                    lg = pb_.tile([128, E], F32, tag="lg", bufs=2,
                                  name=f"lg_{l}_{mt}")
                    nc.vector.tensor_tensor(lg[:], lg_ps[:], rb_sb[:], op=OP.add)
                    m1 = pb_.tile([128, 1], F32, tag="m1", bufs=2,
                                  name=f"m1_{l}_{mt}")
                    nc.vector.reduce_max(m1[:], lg[:], axis=mybir.AxisListType.X)
                    nc.vector.tensor_scalar_sub(lg[:], lg[:], m1[:])
                    ex = pb_.tile([128, E], F32, tag="ex", bufs=2,
                                  name=f"ex_{l}_{mt}")
                    nc.scalar.activation(ex[:], lg[:], AF.Exp)
                    gt = pb_.tile([128, E], F32, tag="gt", bufs=2,
                                  name=f"gt_{l}_{mt}")
                    nc.vector.tensor_scalar(gt[:], ex[:], 1.0, -2.0, OP.is_ge,
                                            OP.mult)
                    nc.vector.tensor_tensor(gt[:], ex[:], gt[:], op=OP.add)
                    m2 = pb_.tile([128, 1], F32, tag="m2", bufs=2,
                                  name=f"m2_{l}_{mt}")
                    nc.vector.reduce_max(m2[:], gt[:], axis=mybir.AxisListType.X)
                    keep = pb_.tile([128, E], F32, tag="keep", bufs=2,
                                    name=f"keep_{l}_{mt}")
                    nc.vector.tensor_scalar(keep[:], ex[:], m2[:], None, OP.is_ge)
                    den = pb_.tile([128, 1], F32, tag="den", bufs=2,
                                   name=f"den_{l}_{mt}")
                    nc.vector.tensor_scalar_add(den[:], m2[:], 1.0)
                    rden = pb_.tile([128, 1], F32, tag="rden", bufs=2,
                                    name=f"rden_{l}_{mt}")
                    nc.vector.reciprocal(rden[:], den[:])
                    nc.vector.tensor_tensor(keep[:], keep[:], ex[:], op=OP.mult)
                    nc.vector.tensor_scalar_mul(keep[:], keep[:], rden[:])
                    nc.sync.dma_start(
                        dbufs[l]["cmb_in"][mt * 128:(mt + 1) * 128, :], keep[:])

            with nc.named_scope(f"l{l}_ag"):
                nc.gpsimd.collective_compute(
                    "AllGather", OP.bypass, replica_groups=rg,
                    ins=[dbufs[l]["cmb_in"][:]], outs=[dbufs[l]["cmb_out"][:]])
                nc.gpsimd.collective_compute(
                    "AllGather", OP.bypass, replica_groups=rg,
                    ins=[dbufs[l]["agf_in"][:]], outs=[dbufs[l]["agf_out"][:]])
                nc.gpsimd.collective_compute(
                    "AllGather", OP.bypass, replica_groups=rg,
                    ins=[dbufs[l]["agt_in"][:]], outs=[dbufs[l]["agt_out"][:]])

            agf = dbufs[l]["agf_out"].rearrange("(c p) t -> c p t", c=NC_)

            lyr.close()

            # ---------------- MoE (dense, this core's expert) --------------
            lyr = ExitStack()
            pm = lyr.enter_context(tc.tile_pool(name=f"pm_{l}", bufs=1))
            pmp = lyr.enter_context(tc.tile_pool(name=f"pmp_{l}", bufs=1, space="PSUM"))

            w2_sb = []
            for kc in range(KH):
                t_ = pw.tile([128, D], BF16, tag=f"w2_{kc}", bufs=1,
                             name=f"w2_{l}_{kc}")
                nc.sync.dma_start(t_[:], w2T[l, kc * 128:(kc + 1) * 128, :])
                w2_sb.append(t_)
            b1_sb = pattw.tile([128, KH], F32, tag="b1", bufs=1, name=f"b1_{l}")
            nc.sync.dma_start(b1_sb[:], b1[l])

            with nc.named_scope(f"l{l}_moe"):
                for stt in range(4):  # 512-token supertiles
                    rhs_tiles = []
                    for kc in range(KD):
                        rhs = pm.tile([128, 512], BF16, tag=f"mrhs{kc}", bufs=2,
                                      name=f"mrhs_{l}_{stt}_{kc}")
                        nc.sync.dma_start(
                            rhs.rearrange("p (b t) -> p b t", b=2),
                            agf[2 * stt:2 * stt + 2,
                                kc * 128:(kc + 1) * 128, :].rearrange(
                                    "b p t -> p b t"))
                        rhs_tiles.append(rhs)
                    h_sb = [None] * KH
                    rev = stt % 2 == 1
                    hg_order = range(8)[::-1] if rev else range(8)
                    for hg in hg_order:  # stream fc1 weights in groups of 4 hc
                        w1g = []
                        for kc in range(KD):
                            wt = pm.tile([128, 512], BF16, tag=f"w1s{kc}", bufs=2,
                                         name=f"w1s_{l}_{stt}_{hg}_{kc}")
                            nc.sync.dma_start(
                                wt[:], w1T[l, kc * 128:(kc + 1) * 128,
                                           hg * 512:(hg + 1) * 512])
                            w1g.append(wt)
                        j_order = range(4)[::-1] if rev else range(4)
                        for j in j_order:
                            hc = hg * 4 + j
                            ps = pmp.tile([128, 512], F32, tag="f_ps", bufs=3,
                                          name=f"fps_{l}_{stt}_{hc}")
                            for kc in range(KD):
                                nc.tensor.matmul(
                                    ps[:], w1g[kc][:, j * 128:(j + 1) * 128],
                                    rhs_tiles[kc][:],
                                    start=(kc == 0), stop=(kc == KD - 1))
                            ht = pm.tile([128, 512], BF16, tag=f"h{hc}", bufs=1,
                                         name=f"h_{l}_{stt}_{hc}")
                            nc.scalar.activation(ht[:], ps[:], AF.Gelu,
                                                 bias=b1_sb[:, hc:hc + 1])
                            h_sb[hc] = ht
                    for mt in range(4):  # 128-token tiles within supertile
                        g = stt * 4 + mt
                        y = pm.tile([128, D], F32, tag="y", bufs=2,
                                    name=f"y_{l}_{g}")
                        for nb in range(2):
                            ps = pmp.tile([128, 512], F32, tag="g_ps", bufs=3,
                                          name=f"gps_{l}_{g}_{nb}")
                            for hc in range(KH):
                                nc.tensor.matmul(
                                    ps[:], h_sb[hc][:, mt * 128:(mt + 1) * 128],
                                    w2_sb[hc][:, nb * 512:(nb + 1) * 512],
                                    start=(hc == 0), stop=(hc == KH - 1))
                            nc.vector.tensor_tensor(
                                y[:, nb * 512:(nb + 1) * 512], ps[:],
                                lc["b2r"][:, nb * 512:(nb + 1) * 512], op=OP.add)
                        xm = pm.tile([128, D], F32, tag="xmtok", bufs=2,
                                     name=f"xmtok_{l}_{g}")
                        nc.sync.dma_start(
                            xm[:], dbufs[l]["agt_out"][g * 128:(g + 1) * 128, :])
                        nc.vector.tensor_tensor(y[:], y[:], xm[:], op=OP.add)
                        cmb8 = pm.tile([128, E], F32, tag="cmb8", bufs=2,
                                       name=f"cmb8_{l}_{g}")
                        nc.sync.dma_start(
                            cmb8[:], dbufs[l]["cmb_out"][g * 128:(g + 1) * 128, :])
                        nc.vector.tensor_tensor(cmb8[:], cmb8[:], sel_sb[:],
                                                op=OP.mult)
                        combg = pm.tile([128, 1], F32, tag="combg", bufs=2,
                                        name=f"combg_{l}_{g}")
                        nc.vector.reduce_sum(combg[:], cmb8[:],
                                             axis=mybir.AxisListType.X)
                        # expert LayerNorm + top-2 combine weight, in place on y
                        mean = pm.tile([128, 1], F32, tag="ln_mean", bufs=4,
                                       name=f"emean_{l}_{g}")
                        nc.vector.reduce_sum(mean[:], y[:],
                                             axis=mybir.AxisListType.X)
                        nc.vector.tensor_scalar_mul(mean[:], mean[:], 1.0 / D)
                        nc.vector.tensor_scalar_sub(y[:], y[:], mean[:])
                        sq = pm.tile([128, D], F32, tag="ln_sq", bufs=1,
                                     name=f"esq_{l}_{g}")
                        vs2 = pm.tile([128, 1], F32, tag="ln_vs", bufs=4,
                                      name=f"evs_{l}_{g}")
                        nc.scalar.activation(sq[:], y[:], AF.Square,
                                             accum_out=vs2[:])
                        std = pm.tile([128, 1], F32, tag="ln_std", bufs=4,
                                      name=f"estd_{l}_{g}")
                        nc.vector.tensor_scalar(std[:], vs2[:], 1.0 / D, EPS,
                                                OP.mult, OP.add)
                        nc.scalar.sqrt(std[:], std[:])
                        rstd = pm.tile([128, 1], F32, tag="ln_rstd", bufs=4,
                                       name=f"erstd_{l}_{g}")
                        nc.vector.reciprocal(rstd[:], std[:])
                        nc.vector.tensor_scalar_mul(y[:], y[:], rstd[:])
                        nc.vector.tensor_tensor(y[:], y[:], lc["egr"][:],
                                                op=OP.mult)
                        nc.vector.tensor_tensor(y[:], y[:], lc["ebr"][:],
                                                op=OP.add)
                        yb = pm.tile([128, D], BF16, tag="yb", bufs=1,
                                     name=f"yb_{l}_{g}")
                        nc.vector.tensor_scalar(yb[:], y[:], combg[:], None,
                                                OP.mult)
                        nc.sync.dma_start(
                            dbufs[l]["rs2_in"][g * 128:(g + 1) * 128, :], yb[:])

            with nc.named_scope(f"l{l}_rs2"):
                nc.gpsimd.collective_compute(
                    "ReduceScatter", OP.add, replica_groups=rg,
                    ins=[dbufs[l]["rs2_in"][:]], outs=[dbufs[l]["rs2_out"][:]])

            # ---------------- LN2 -> next x ------------------------------
            xout = new_xres(f"xout{l}")
            with nc.named_scope(f"l{l}_ln2"):
                for mt in range(2):
                    tinb = pm.tile([128, D], BF16, tag="rs2tb", bufs=1,
                                   name=f"rs2tb_{l}_{mt}")
                    nc.sync.dma_start(
                        tinb[:], dbufs[l]["rs2_out"][mt * 128:(mt + 1) * 128, :])
                    tin = pm.tile([128, D], F32, tag="rs2t", bufs=1,
                                  name=f"rs2t_{l}_{mt}")
                    nc.vector.tensor_tensor(tin[:], tinb[:], xmid[mt][:], op=OP.add)
                    _ln_tile(nc, pm, tin[:], lc["ln2g"][:], lc["ln2b"][:],
                             xout[mt][:])
                    for kc in range(KD):
                        pt = pmp.tile([128, 128], F32, tag="tr2_ps", bufs=2,
                                      name=f"tr2ps_{l}_{mt}_{kc}")
                        nc.tensor.transpose(
                            pt[:], xout[mt][:, kc * 128:(kc + 1) * 128], ident_f[:])
                        tb = pm.tile([128, 128], BF16, tag="tr2_sb", bufs=2,
                                     name=f"tr2sb_{l}_{mt}_{kc}")
                        nc.scalar.copy(tb[:], pt[:])
                        nc.sync.dma_start(
                            dbufs[l]["agx_in"][kc * 128:(kc + 1) * 128,
                                               mt * 128:(mt + 1) * 128], tb[:])

            with nc.named_scope(f"l{l}_agx"):
                nc.gpsimd.collective_compute(
                    "AllGather", OP.bypass, replica_groups=rg,
                    ins=[dbufs[l]["agx_in"][:]], outs=[dbufs[l]["agx_out"][:]])

            if debug:
                nc.sync.dma_start(dbg[f"agf_{l}"][:], dbufs[l]["agf_out"][:])
                nc.sync.dma_start(dbg[f"rs1o_{l}"][:], dbufs[l]["rs1_out"][:])
                nc.sync.dma_start(dbg[f"agt_{l}"][:], dbufs[l]["agt_out"][:])
                nc.sync.dma_start(dbg[f"comb_{l}"][:], dbufs[l]["cmb_out"][:])
                nc.sync.dma_start(dbg[f"rs2o_{l}"][:], dbufs[l]["rs2_out"][:])

            xres = xout
            lyr.close()

        # ---------------- vocab projection (column-parallel) --------------
        es.close()  # release weight/activation pools so projT fits
        vx = ExitStack()
        pv = vx.enter_context(tc.tile_pool(name="pv", bufs=1))
        pvp = vx.enter_context(tc.tile_pool(name="pvp", bufs=1, space="PSUM"))
        with nc.named_scope("vocab"):
            pj_sb = []
            for kc in range(KD):
                t_ = pv.tile([128, VS], BF16, tag=f"pj{kc}", bufs=1,
                             name=f"pj_{kc}")
                nc.sync.dma_start(t_[:], projT[kc * 128:(kc + 1) * 128, :])
                pj_sb.append(t_)
            pb_sb = pv.tile([128, VS], F32, tag="pb", bufs=1, name="pb_sb")
            nc.sync.dma_start(pb_sb[:], pbr[:])
            agd = xblocks[L]
            for g in range(T // 128):
                blk, off = g // 2, (g % 2) * 128
                lhs_tiles = []
                for kc in range(KD):
                    xt = pv.tile([128, 128], BF16, tag=f"vxt{kc}", bufs=2,
                                 name=f"vxt_{g}_{kc}")
                    nc.sync.dma_start(
                        xt[:], agd[blk, kc * 128:(kc + 1) * 128, off:off + 128])
                    lhs_tiles.append(xt)
                for vb in range(8):
                    ps = pvp.tile([128, 500], F32, tag="v_ps", bufs=4,
                                  name=f"vps_{g}_{vb}")
                    for kc in range(KD):
                        nc.tensor.matmul(
                            ps[:], lhs_tiles[kc][:],
                            pj_sb[kc][:, vb * 500:(vb + 1) * 500],
                            start=(kc == 0), stop=(kc == KD - 1))
                    lo = pv.tile([128, 500], F32, tag="lo", bufs=4,
                                 name=f"lo_{g}_{vb}")
                    nc.vector.tensor_tensor(
                        lo[:], ps[:], pb_sb[:, vb * 500:(vb + 1) * 500], op=OP.add)
                    nc.sync.dma_start(
                        logits[g * 128:(g + 1) * 128, vb * 500:(vb + 1) * 500],
                        lo[:])
        vx.close()
        esd.close()

    nc.compile()
    return nc


# ---------------------------------------------------------------- host side

def _bf16(a):
    return np.ascontiguousarray(np.asarray(a).astype(ml_dtypes.bfloat16))


def _f32(a):
    return np.ascontiguousarray(np.asarray(a).astype(np.float32))


def _rep(v):
    """Replicate a [N] vector across 128 partitions -> [128, N]."""
    v = np.asarray(v, np.float32)
    return np.ascontiguousarray(np.broadcast_to(v, (128, v.shape[0])))


def _pos_encoding(S_, D_):
    pos = np.arange(S_, dtype=np.float32)[:, None]
    div = np.exp(np.arange(0, D_, 2, dtype=np.float32) * (-math.log(10000.0) / D_))
    pe = np.zeros((S_, D_), dtype=np.float32)
    pe[:, 0::2] = np.sin(pos * div)
    pe[:, 1::2] = np.cos(pos * div)
    return pe


def make_in_maps(input_ids, emb, qkv_w, qkv_b, out_w, out_b, ln1_g, ln1_b,
                 router_w, router_b, fc1_w, fc1_b, fc2_w, fc2_b, eln_g, eln_b,
                 ln2_g, ln2_b, proj_w, proj_b):
    input_ids = np.asarray(input_ids)
    emb = np.asarray(emb)
    qkv_w, qkv_b = np.asarray(qkv_w), np.asarray(qkv_b)
    out_w, out_b = np.asarray(out_w), np.asarray(out_b)
    ln1_g, ln1_b = np.asarray(ln1_g), np.asarray(ln1_b)
    router_w, router_b = np.asarray(router_w), np.asarray(router_b)
    fc1_w, fc1_b = np.asarray(fc1_w), np.asarray(fc1_b)
    fc2_w, fc2_b = np.asarray(fc2_w), np.asarray(fc2_b)
    eln_g, eln_b = np.asarray(eln_g), np.asarray(eln_b)
    ln2_g, ln2_b = np.asarray(ln2_g), np.asarray(ln2_b)
    proj_w, proj_b = np.asarray(proj_w), np.asarray(proj_b)

    ids = input_ids.reshape(T)
    pe = _pos_encoding(S, D)
    x0 = (emb[ids] * math.sqrt(D) + np.tile(pe, (B, 1))).astype(np.float32)
    x0Tb = _bf16(x0.reshape(NC_, TS, D).transpose(0, 2, 1))

    # shared (replicated) tensors
    shared = {
        "x0Tb": x0Tb,
        "rwT32": _f32(router_w.transpose(0, 2, 1)),
        "rb": _f32(np.stack([_rep(router_b[l]) for l in range(L)])),
        "outb": _f32(np.stack([_rep(out_b[l]) for l in range(L)])),
        "ln1g": _f32(np.stack([_rep(ln1_g[l]) for l in range(L)])),
        "ln1b": _f32(np.stack([_rep(ln1_b[l]) for l in range(L)])),
        "ln2g": _f32(np.stack([_rep(ln2_g[l]) for l in range(L)])),
        "ln2b": _f32(np.stack([_rep(ln2_b[l]) for l in range(L)])),
    }

    in_maps = []
    for c in range(NC_):
        m = dict(shared)
        m["x0_sh"] = _f32(x0[c * TS:(c + 1) * TS])
        wq = np.empty((L, D, 3 * 128), np.float32)
        qb = np.empty((L, 128, 3), np.float32)
        for l in range(L):
            for r in range(3):
                rows = slice(r * D + c * 128, r * D + (c + 1) * 128)
                wq[l, :, r * 128:(r + 1) * 128] = qkv_w[l, rows, :].T
                qb[l, :, r] = qkv_b[l, rows]
        m["wqkvT"] = _bf16(wq)
        m["qkvb"] = _f32(qb)
        m["owT"] = _bf16(out_w[:, :, c * 128:(c + 1) * 128].transpose(0, 2, 1))
        sel_ = np.zeros((128, E), np.float32)
        sel_[:, c] = 1.0
        m["sel"] = sel_
        m["w1T"] = _bf16(fc1_w[:, c].transpose(0, 2, 1))
        m["b1"] = _f32(fc1_b[:, c].reshape(L, KH, 128).transpose(0, 2, 1))
        m["w2T"] = _bf16(fc2_w[:, c].transpose(0, 2, 1))
        m["b2r"] = _f32(np.stack([_rep(fc2_b[l, c]) for l in range(L)]))
        m["egr"] = _f32(np.stack([_rep(eln_g[l, c]) for l in range(L)]))
        m["ebr"] = _f32(np.stack([_rep(eln_b[l, c]) for l in range(L)]))
        m["projT"] = _bf16(proj_w[c * VS:(c + 1) * VS].T)
        m["pbr"] = _f32(_rep(proj_b[c * VS:(c + 1) * VS]))
        in_maps.append(m)
    return in_maps


def get_compiled(debug=False):
    global _COMPILED
    if _COMPILED is None:
        _COMPILED = build_program(debug=debug)
    return _COMPILED


def kernel(_trace=False, _debug=False, **inputs):
    nc = get_compiled(debug=_debug)
    in_maps = make_in_maps(**inputs)
    res = run_bass_kernel_spmd(nc, in_maps, core_ids=list(range(NC_)),
                               trace=_trace)
    out = np.concatenate([res.results[c]["logits"] for c in range(NC_)], axis=1)
    out = out.reshape(B, S, V).astype(np.float32)
    kernel.last_exec_time_ns = res.exec_time_ns
    kernel.last_results = res.results
    kernel.last_scopes = res.per_core_scope_times
    return out
